# revision 1
# baseline (speedup 1.0000x reference)
"""Trainium2 Bass kernel for MultiHeadedSelfAttention with Shaw relative
position embeddings (clipped, R=64), sharded over 8 NeuronCores.

Sharding: core c handles batch b = c//4 and head group g = c%4 (4 heads).
Each core computes a partial output  ctx_g @ W_out[256g:256g+256]  for its
batch; the host sums the 4 partials per batch and adds b_out.
"""
import sys

sys.path.insert(0, "/opt/trn_rl_repo")

import numpy as np

B, S, D, H, RR, VOC = 2, 2048, 1024, 16, 64, 129
HD = 64              # head dim
NH = 4               # heads per core
N_CORES = 8
NT = S // 128        # 16 q-tiles of 128
NKT = S // 128       # 16 k-tiles
IMW = 512            # qrel image width (clip-padded)
IMWW = 384           # attn/cumsum image width (W-zone grid)
SCALE = 0.125        # 1/sqrt(64)

_cache = {}


def _regions(t):
    """W-zone bounds for q-tile t."""
    i0 = 128 * t
    wlo = max(0, i0 - 128)
    whi = min(S, i0 + 256)
    return i0, wlo, whi


def _build():
    import concourse.bass as bass
    import concourse.mybir as mybir
    import concourse.tile as tile
    from concourse import bacc
    from concourse.masks import make_identity
    from contextlib import ExitStack

    F32 = mybir.dt.float32
    F32R = mybir.dt.float32r
    F16 = mybir.dt.float16
    AP = bass.AP
    AF = mybir.ActivationFunctionType
    ALU = mybir.AluOpType

    nc = bacc.Bacc("TRN2", target_bir_lowering=False, debug=False,
                   num_devices=N_CORES)

    # ---------------- DRAM I/O ----------------
    xT = nc.dram_tensor("xT", [D, S], F32, kind="ExternalInput").ap()
    wq = nc.dram_tensor("wq", [D, 256], F32, kind="ExternalInput").ap()
    wk = nc.dram_tensor("wk", [D, 256], F32, kind="ExternalInput").ap()
    wv = nc.dram_tensor("wv", [D, 256], F32, kind="ExternalInput").ap()
    bq = nc.dram_tensor("bq", [128, 2], F32, kind="ExternalInput").ap()
    bk = nc.dram_tensor("bk", [128, 2], F32, kind="ExternalInput").ap()
    bv = nc.dram_tensor("bv", [128, 2], F32, kind="ExternalInput").ap()
    r01 = nc.dram_tensor("r01", [128, 2], F32, kind="ExternalInput").ap()
    relk = nc.dram_tensor("relk", [128, 132], F16, kind="ExternalInput").ap()
    rvm = nc.dram_tensor("rvm", [128, 64], F16, kind="ExternalInput").ap()
    rvl = nc.dram_tensor("rvl", [1, 64], F16, kind="ExternalInput").ap()
    wout = nc.dram_tensor("wout", [128, 2, 1024], F16, kind="ExternalInput").ap()
    out = nc.dram_tensor("out", [S, D], F32, kind="ExternalOutput").ap()

    # DRAM scratch images (per head, per q-tile blocks)
    imgq_t = nc.dram_tensor("imgq", [NH * NT * 128 * IMW], F16)
    imgw_t = nc.dram_tensor("imgw", [NH * NT * 128 * IMWW], F16)
    imgc_t = nc.dram_tensor("imgc", [NH * NT * 128 * IMWW], F16)

    with tile.TileContext(nc) as tc, ExitStack() as ctx:
        # ---------------- persistent pools ----------------
        pp = ctx.enter_context(tc.tile_pool(name="persist", bufs=1))
        qkT = []   # per pair: qT16, kW16, kL16, kR16  [128, S] fp16
        for pair in range(2):
            qkT.append({
                "q": pp.tile([128, S], F16, tag=f"qT{pair}", name=f"qT{pair}"),
                "W": pp.tile([128, S], F16, tag=f"kW{pair}", name=f"kW{pair}"),
                "L": pp.tile([128, S], F16, tag=f"kL{pair}", name=f"kL{pair}"),
                "R": pp.tile([128, S], F16, tag=f"kR{pair}", name=f"kR{pair}"),
            })
        v16 = pp.tile([128, NKT, 256], F16, tag="v16", name="v16")
        relk_sb = pp.tile([128, 132], F16, tag="relk", name="relk")
        rvm_sb = pp.tile([128, 64], F16, tag="rvm", name="rvm")
        rvl_sb = pp.tile([1, 64], F16, tag="rvl", name="rvl")
        wout_sb = pp.tile([128, 2, 1024], F16, tag="wout", name="wout")
        bq_sb = pp.tile([128, 2], F32, tag="bq", name="bq")
        bk_sb = pp.tile([128, 2], F32, tag="bk", name="bk")
        bv_sb = pp.tile([128, 2], F32, tag="bv", name="bv")
        r01_sb = pp.tile([128, 2], F32, tag="r01", name="r01")
        ones1 = pp.tile([1, 128], F16, tag="ones1", name="ones1")
        zeros16 = pp.tile([128, 128], F16, tag="zeros16", name="zeros16")
        # attnT pool: per head [128 k, kt, 512 q-group]  fp16
        attnT = [pp.tile([128, NKT, 512], F16, tag=f"attnT{h}", name=f"attnT{h}") for h in range(NH)]
        arelT = [pp.tile([128, 2, 512], F16, tag=f"arelT{h}", name=f"arelT{h}") for h in range(NH)]

        nc.sync.dma_start(relk_sb[:], relk)
        nc.sync.dma_start(rvm_sb[:], rvm)
        nc.sync.dma_start(rvl_sb[:], rvl)
        nc.sync.dma_start(wout_sb[:], wout)
        nc.sync.dma_start(bq_sb[:], bq)
        nc.sync.dma_start(bk_sb[:], bk)
        nc.sync.dma_start(bv_sb[:], bv)
        nc.sync.dma_start(r01_sb[:], r01)
        nc.gpsimd.memset(ones1[:], 1.0)
        nc.gpsimd.memset(zeros16[:], 0.0)

        # ---------------- phase 1: projections ----------------
        with tc.tile_pool(name="p1", bufs=1) as p1, \
             tc.tile_pool(name="p1ps", bufs=2, space="PSUM") as p1ps:
            xT_sb = p1.tile([128, 8, S], F32R, tag="xT", name="xT")
            wq_sb = p1.tile([128, 8, 256], F32R, tag="wq", name="wq")
            wk_sb = p1.tile([128, 8, 256], F32R, tag="wk", name="wk")
            wv_sb = p1.tile([128, 8, 256], F32R, tag="wv", name="wv")
            nc.sync.dma_start(xT_sb[:], xT.rearrange("(c p) s -> p c s", p=128).bitcast(F32R))
            nc.sync.dma_start(wq_sb[:], wq.rearrange("(c p) n -> p c n", p=128).bitcast(F32R))
            nc.sync.dma_start(wk_sb[:], wk.rearrange("(c p) n -> p c n", p=128).bitcast(F32R))
            nc.sync.dma_start(wv_sb[:], wv.rearrange("(c p) n -> p c n", p=128).bitcast(F32R))

            # q, k (transposed layout [col, s]) per pair
            for pair in range(2):
                for sc in range(4):  # s-chunks of 512
                    ps_q = p1ps.tile([128, 512], F32, tag="p1q", name="p1q")
                    ps_k = p1ps.tile([128, 512], F32, tag="p1k", name="p1k")
                    for dk in range(8):
                        nc.tensor.matmul(
                            ps_q[:], wq_sb[:, dk, 128 * pair:128 * pair + 128],
                            xT_sb[:, dk, 512 * sc:512 * sc + 512],
                            start=(dk == 0), stop=(dk == 7))
                        nc.tensor.matmul(
                            ps_k[:], wk_sb[:, dk, 128 * pair:128 * pair + 128],
                            xT_sb[:, dk, 512 * sc:512 * sc + 512],
                            start=(dk == 0), stop=(dk == 7))
                    cs = slice(512 * sc, 512 * sc + 512)
                    # qT16 = (psum + bq) * SCALE ; kW16 = psum + bk
                    nc.vector.tensor_scalar(
                        qkT[pair]["q"][:, cs], ps_q[:], bq_sb[:, pair:pair + 1],
                        SCALE, op0=ALU.add, op1=ALU.mult)
                    nc.vector.tensor_scalar_add(
                        qkT[pair]["W"][:, cs], ps_k[:], bk_sb[:, pair:pair + 1])
                    nc.vector.tensor_scalar_add(
                        qkT[pair]["L"][:, cs], qkT[pair]["W"][:, cs],
                        r01_sb[:, 0:1])
                    nc.vector.tensor_scalar_add(
                        qkT[pair]["R"][:, cs], qkT[pair]["W"][:, cs],
                        r01_sb[:, 1:2])

            # v (natural layout [s, col])
            for st in range(NT):
                ps_v = p1ps.tile([128, 256], F32, tag="p1v", name="p1v")
                for dk in range(8):
                    nc.tensor.matmul(
                        ps_v[:], xT_sb[:, dk, 128 * st:128 * st + 128],
                        wv_sb[:, dk, :], start=(dk == 0), stop=(dk == 7))
                nc.vector.tensor_copy(v16[:, st, :], ps_v[:])

        # ---------------- phase 2: attention ----------------
        p2 = ctx.enter_context(tc.tile_pool(name="p2", bufs=1))
        scps = ctx.enter_context(tc.tile_pool(name="scps", bufs=2, space="PSUM"))
        avps = ctx.enter_context(tc.tile_pool(name="avps", bufs=1, space="PSUM"))
        smps = ctx.enter_context(tc.tile_pool(name="smps", bufs=2, space="PSUM"))
        expp = ctx.enter_context(tc.tile_pool(name="expp", bufs=5))
        smal = ctx.enter_context(tc.tile_pool(name="smal", bufs=4))

        def img_base(handle, h, t, w):
            return (h * NT + t) * 128 * w

        for go in range(4):            # q512 groups
            # fp16 reciprocals laid out [1, head*512 + q] for K=1 broadcast
            recipT16 = smal.tile([1, 4 * 512], F16, tag="recipT16", name="recipT16")
            for tq in range(4):        # q128 tiles within group
                t = 4 * go + tq
                i0, wlo, whi = _regions(t)
                wlen = whi - wlo
                moff = wlo - (i0 - 128)      # attnW grid offset (0 or 128)
                recips = smal.tile([128, 4], F32, tag="recips", name="recips")
                for pair in range(2):
                    # ---- Qrel (both heads, row-tiled) -> qrel image
                    qrps = [smps.tile([128, 132], F32, tag="sm", name="sm") for _ in range(2)]
                    for h01 in range(2):
                        rs = slice(64 * h01, 64 * h01 + 64)
                        nc.tensor.matmul(
                            qrps[h01][:], qkT[pair]["q"][rs, 128 * t:128 * t + 128],
                            relk_sb[rs, :], start=True, stop=True)
                    for h01 in range(2):
                        h = 2 * pair + h01
                        qrelpad = smal.tile([128, IMW], F16, tag="qrelpad", name="qrelpad")
                        nc.vector.tensor_copy(qrelpad[:, 192:321], qrps[h01][:, 0:129])
                        nc.vector.tensor_copy(
                            qrelpad[:, 0:192],
                            qrps[h01][:, 0:1].broadcast_to([128, 192]))
                        nc.vector.tensor_copy(
                            qrelpad[:, 321:IMW],
                            qrps[h01][:, 128:129].broadcast_to([128, IMW - 321]))
                        b = img_base(imgq_t, h, t, IMW)
                        nc.sync.dma_start(
                            AP(imgq_t, b, [[IMW, 128], [1, IMW]]), qrelpad[:])

                    # ---- scores + band + exp, per 512-col chunk
                    bands = {}
                    exps = {}
                    accs = {}
                    for h01 in range(2):
                        h = 2 * pair + h01
                        bq_ = img_base(imgq_t, h, t, IMW)
                        band = smal.tile([128, wlen], F16, tag="band", name="band")
                        nc.sync.dma_start(
                            band[:],
                            AP(imgq_t, bq_ + 256 + wlo - i0, [[IMW - 1, 128], [1, wlen]]))
                        bands[h01] = band
                        exps[h01] = expp.tile([128, S], F16, tag="exp", name="exp")
                        accs[h01] = smal.tile([128, 4], F32, tag="acc", name="acc")

                    for c in range(4):   # 512-col chunks
                        clo, chi = 512 * c, 512 * c + 512
                        sc_ps = [scps.tile([128, 512], F32, tag=f"sc{h01}", name=f"sc{h01}")
                                 for h01 in range(2)]
                        # region pieces within this chunk
                        pieces = []
                        for rlo, rhi, key in ((0, wlo, "L"), (wlo, whi, "W"),
                                              (whi, S, "R")):
                            lo, hi = max(rlo, clo), min(rhi, chi)
                            if lo < hi:
                                pieces.append((lo, hi, key))
                        for h01 in range(2):
                            rs = slice(64 * h01, 64 * h01 + 64)
                            for lo, hi, key in pieces:
                                nc.tensor.matmul(
                                    sc_ps[h01][:, lo - clo:hi - clo],
                                    qkT[pair]["q"][rs, 128 * t:128 * t + 128],
                                    qkT[pair][key][rs, lo:hi],
                                    start=True, stop=True)
                        # band add (W-zone part of chunk)
                        blo, bhi = max(wlo, clo), min(whi, chi)
                        for h01 in range(2):
                            if blo < bhi:
                                nc.vector.tensor_add(
                                    sc_ps[h01][:, blo - clo:bhi - clo],
                                    sc_ps[h01][:, blo - clo:bhi - clo],
                                    bands[h01][:, blo - wlo:bhi - wlo])
                            nc.scalar.activation(
                                exps[h01][:, clo:chi], sc_ps[h01][:], AF.Exp,
                                accum_out=accs[h01][:, c:c + 1])

                    # ---- per-head epilogue: cumsum, images, Arel, recip
                    for h01 in range(2):
                        h = 2 * pair + h01
                        ex = exps[h01]
                        acc = accs[h01]
                        Cpad = smal.tile([128, IMWW], F16, tag="Cpad", name="Cpad")
                        C16 = Cpad[:, moff:moff + wlen]
                        nc.vector.tensor_tensor_scan(
                            C16, ex[:, wlo:whi], ex[:, wlo:whi], 0.0,
                            op0=ALU.add, op1=ALU.bypass)
                        bw = img_base(imgw_t, h, t, IMWW)
                        bc = img_base(imgc_t, h, t, IMWW)
                        nc.sync.dma_start(
                            AP(imgw_t, bw + moff, [[IMWW, 128], [1, wlen]]),
                            ex[:, wlo:whi])
                        if moff > 0:   # t == 0: zero guards on the left
                            nc.sync.dma_start(
                                AP(imgw_t, bw, [[IMWW, 128], [1, moff]]),
                                zeros16[:, 0:moff])
                            nc.vector.tensor_copy(Cpad[:, 0:moff],
                                                  zeros16[:, 0:moff])
                        if moff + wlen < IMWW:  # t == 15: right guards
                            gl = IMWW - (moff + wlen)
                            nc.sync.dma_start(
                                AP(imgw_t, bw + moff + wlen, [[IMWW, 128], [1, gl]]),
                                zeros16[:, 0:gl])
                            # cumsum saturates: right guard = row total
                            nc.vector.tensor_copy(
                                Cpad[:, moff + wlen:IMWW],
                                C16[:, wlen - 1:wlen].broadcast_to([128, gl]))
                        nc.sync.dma_start(
                            AP(imgc_t, bc, [[IMWW, 128], [1, IMWW]]), Cpad[:])

                        # D = sum of chunk accums
                        Dt = smal.tile([128, 1], F32, tag="Dt", name="Dt")
                        nc.vector.tensor_reduce(
                            Dt[:], acc[:], axis=mybir.AxisListType.X, op=ALU.add)

                        # S_L from chunk accums minus in-chunk W mass
                        sL = smal.tile([128, 1], F32, tag="sL", name="sL")
                        if wlo == 0:
                            nc.vector.memset(sL[:], 0.0)
                        else:
                            nfull = wlo // 512
                            r = wlo % 512
                            terms = [acc[:, c:c + 1] for c in range(nfull)]
                            if nfull == 1:
                                nc.vector.tensor_copy(sL[:], terms[0])
                            elif nfull > 1:
                                nc.vector.tensor_reduce(
                                    sL[:], acc[:, 0:nfull],
                                    axis=mybir.AxisListType.X, op=ALU.add)
                            if r > 0:
                                # straddle chunk nfull: L part = a - Wmass
                                mhi = min(512 - r, wlen)   # W cols [0, mhi) in chunk
                                tmp = smal.tile([128, 1], F32, tag="tmp1", name="tmp1")
                                nc.vector.tensor_tensor(
                                    tmp[:], acc[:, nfull:nfull + 1],
                                    C16[:, mhi - 1:mhi], op=ALU.subtract)
                                if nfull == 0:
                                    nc.vector.tensor_copy(sL[:], tmp[:])
                                else:
                                    nc.vector.tensor_add(sL[:], sL[:], tmp[:])

                        # skew reads: arel central + c0/c1
                        arel16 = smal.tile([128, 256], F16, tag="arel16", name="arel16")
                        nc.vector.memset(arel16[:, 129:256], 0.0)
                        nc.sync.dma_start(
                            arel16[:, 1:128],
                            AP(imgw_t, bw + 65, [[IMWW + 1, 128], [1, 127]]))
                        c0 = smal.tile([128, 2], F16, tag="c01", name="c01")
                        nc.sync.dma_start(
                            c0[:, 0:1],
                            AP(imgc_t, bc + 64, [[IMWW + 1, 128], [1, 1]]))
                        nc.sync.dma_start(
                            c0[:, 1:2],
                            AP(imgc_t, bc + 191, [[IMWW + 1, 128], [1, 1]]))
                        nc.vector.tensor_tensor(
                            arel16[:, 0:1], sL[:], c0[:, 0:1], op=ALU.add)
                        tmp2 = smal.tile([128, 1], F32, tag="tmp2", name="tmp2")
                        nc.vector.tensor_tensor(
                            tmp2[:], Dt[:], sL[:], op=ALU.subtract)
                        nc.vector.tensor_tensor(
                            arel16[:, 128:129], tmp2[:], c0[:, 1:2],
                            op=ALU.subtract)

                        # transposes
                        nc.sync.dma_start_transpose(
                            attnT[h][:, :, 128 * tq:128 * tq + 128], ex[:])
                        nc.sync.dma_start_transpose(
                            arelT[h][:, :, 128 * tq:128 * tq + 128], arel16[:])

                        # reciprocal of D into recips col h
                        nc.vector.reciprocal(recips[:, h:h + 1], Dt[:])

                # scatter recips [128 q, 4 h] -> recipT16 [1, 512h + 128tq + q]
                recips16 = smal.tile([128, 4], F16, tag="recips16", name="recips16")
                nc.vector.tensor_copy(recips16[:], recips[:])
                for h in range(4):
                    o = 512 * h + 128 * tq
                    nc.sync.dma_start(recipT16[0:1, o:o + 128],
                                      recips16[:, h:h + 1])

            # ---------------- per-group AV + rel-v + normalize + out ----
            ctx16 = {}
            for pair in range(2):
                # broadcast recip rows via K=1 outer product (fp16)
                bc_ps = smps.tile([128, 512], F32, tag="sm", name="sm")
                for h01 in range(2):
                    h = 2 * pair + h01
                    nc.tensor.matmul(
                        bc_ps[64 * h01:64 * h01 + 64, :], ones1[0:1, 0:64],
                        recipT16[0:1, 512 * h:512 * h + 512], start=True,
                        stop=True, tile_position=(0, 64 * h01))
                rbc = smal.tile([128, 512], F32, tag="rbc", name="rbc")
                nc.vector.tensor_copy(rbc[:], bc_ps[:])

                ctx_ps = avps.tile([128, 512], F32, tag=f"av{pair}", name=f"av{pair}")
                for h01 in range(2):
                    h = 2 * pair + h01
                    cs = slice(64 * h01, 64 * h01 + 64)
                    tp = (0, 64 * h01)
                    for kt in range(NKT):
                        nc.tensor.matmul(
                            ctx_ps[cs, :], v16[:, kt, 64 * h:64 * h + 64],
                            attnT[h][:, kt, :], start=(kt == 0), stop=False,
                            tile_position=tp)
                    nc.tensor.matmul(
                        ctx_ps[cs, :], rvm_sb[:, :], arelT[h][:, 0, :],
                        start=False, stop=False, tile_position=tp)
                    nc.tensor.matmul(
                        ctx_ps[cs, :], rvl_sb[0:1, :], arelT[h][0:1, 1, :],
                        start=False, stop=True, tile_position=tp)

                ct = p2.tile([128, 512], F16, tag=f"ctx16_{pair}", name=f"ctx16_{pair}")
                nc.vector.tensor_tensor(ct[:], ctx_ps[:], rbc[:], op=ALU.mult)
                nc.vector.tensor_scalar_add(ct[:], ct[:], bv_sb[:, pair:pair + 1])
                ctx16[pair] = ct

            # ---- output projection for this q512 group
            for tq in range(4):
                out_sb = smal.tile([128, 1024], F32, tag="out_sb", name="out_sb")
                for nch in range(2):
                    op_ps = smps.tile([128, 512], F32, tag="sm", name="sm")
                    for pair in range(2):
                        nc.tensor.matmul(
                            op_ps[:], ctx16[pair][:, 128 * tq:128 * tq + 128],
                            wout_sb[:, pair, 512 * nch:512 * nch + 512],
                            start=(pair == 0), stop=(pair == 1))
                    nc.vector.tensor_copy(
                        out_sb[:, 512 * nch:512 * nch + 512], op_ps[:])
                r0_ = 512 * go + 128 * tq
                nc.sync.dma_start(out[r0_:r0_ + 128, :], out_sb[:])

    nc.compile()
    return nc


def get_nc():
    if "nc" not in _cache:
        _cache["nc"] = _build()
    return _cache["nc"]


def shard_inputs(inputs):
    """Build per-core input maps from full inputs (layout prep only)."""
    x = np.asarray(inputs["x"], np.float32)
    W_qkv = np.asarray(inputs["W_qkv"], np.float32)
    b_qkv = np.asarray(inputs["b_qkv"], np.float32)
    W_out = np.asarray(inputs["W_out"], np.float32)
    rk = np.asarray(inputs["rel_emb_k"], np.float32)
    rv = np.asarray(inputs["rel_emb_v"], np.float32)

    Wq, Wk, Wv = W_qkv[:, 0:D], W_qkv[:, D:2 * D], W_qkv[:, 2 * D:3 * D]
    bqf, bkf, bvf = b_qkv[0:D], b_qkv[D:2 * D], b_qkv[2 * D:3 * D]

    relk_host = np.zeros((128, 132), np.float16)
    relk_host[0:64, 0:129] = rk.T.astype(np.float16)
    relk_host[64:128, 0:129] = rk.T.astype(np.float16)
    rvm_host = rv[0:128].astype(np.float16)
    rvl_host = rv[128:129].astype(np.float16)
    r0 = np.tile(rk[0], 2).reshape(128, 1)
    r1 = np.tile(rk[128], 2).reshape(128, 1)
    r01_host = np.concatenate([r0, r1], 1).astype(np.float32)

    in_maps = []
    for c in range(N_CORES):
        b, g = c // 4, c % 4
        cols = slice(256 * g, 256 * g + 256)
        m = {
            "xT": np.ascontiguousarray(x[b].T),
            "wq": np.ascontiguousarray(Wq[:, cols]),
            "wk": np.ascontiguousarray(Wk[:, cols]),
            "wv": np.ascontiguousarray(Wv[:, cols]),
            "bq": np.ascontiguousarray(bqf[cols].reshape(2, 128).T),
            "bk": np.ascontiguousarray(bkf[cols].reshape(2, 128).T),
            "bv": np.ascontiguousarray(bvf[cols].reshape(2, 128).T),
            "r01": r01_host,
            "relk": relk_host,
            "rvm": rvm_host,
            "rvl": rvl_host,
            "wout": np.ascontiguousarray(
                W_out[cols].reshape(2, 128, 1024).transpose(1, 0, 2)
            ).astype(np.float16),
        }
        in_maps.append(m)
    return in_maps


def unshard_outputs(results, inputs):
    b_out = np.asarray(inputs["b_out"], np.float32)
    out = np.zeros((B, S, D), np.float32)
    for c in range(N_CORES):
        out[c // 4] += results[c]["out"]
    out += b_out[None, None, :]
    return out


def kernel(**inputs):
    from concourse import bass_utils
    nc = get_nc()
    in_maps = shard_inputs(inputs)
    res = bass_utils.run_bass_kernel_spmd(nc, in_maps, list(range(N_CORES)))
    return unshard_outputs(res.results, inputs)


if __name__ == "__main__":
    import json
    rng = np.random.default_rng(0)
    demo = {
        "x": rng.standard_normal((B, S, D)).astype(np.float32),
        "W_qkv": (rng.standard_normal((D, 3 * D)) * 0.02).astype(np.float32),
        "b_qkv": np.zeros(3 * D, np.float32),
        "W_out": (rng.standard_normal((D, D)) * 0.02).astype(np.float32),
        "b_out": np.zeros(D, np.float32),
        "rel_emb_k": (rng.standard_normal((VOC, HD)) * 0.02).astype(np.float32),
        "rel_emb_v": (rng.standard_normal((VOC, HD)) * 0.02).astype(np.float32),
    }
    o = kernel(**demo)
    print(o.shape, float(np.abs(o).max()))



# revision 13
# speedup vs baseline: 1.4494x; 1.4494x over previous
"""Trainium2 Bass kernel for MultiHeadedSelfAttention with Shaw relative
position embeddings (clipped, R=64), sharded over 8 NeuronCores.

Sharding: core c handles batch b = c//4 and head group g = c%4 (4 heads).
Each core computes a partial output  ctx_g @ W_out[256g:256g+256]  for its
batch; the host sums the 4 partials per batch and adds b_out.

v2: phase-2 restructured for fewer DMA instructions (the v1 bottleneck was
the SP sequencer issuing ~550 small DMAs at ~0.7us fixed cost each):
  - qrel pad built by matmul against a host-padded relk_pad table
  - per-(t,pair) image writes / batched per-(pair[,go]) image reads
  - exp in 1024-wide chunks; sL via D - C_W - massR (massR = DVE reduce)
  - fused 2-head transposes; scatter/gather DMAs moved to gpsimd (SWDGE)
"""
import sys

sys.path.insert(0, "/opt/trn_rl_repo")

import numpy as np

B, S, D, H, RR, VOC = 2, 2048, 1024, 16, 64, 129
HD = 64              # head dim
NH = 4               # heads per core
N_CORES = 8
NT = S // 128        # 16 q-tiles of 128
IMW = 512            # qrel image width (clip-padded)
IMWW = 384           # attn/cumsum image width (W-zone grid)
SCALE = 0.125        # 1/sqrt(64)
CW = 1024            # exp chunk width

_cache = {}


def _regions(t):
    """W-zone bounds for q-tile t."""
    i0 = 128 * t
    wlo = max(0, i0 - 128)
    whi = min(S, i0 + 256)
    return i0, wlo, whi


def _build():
    import concourse.bass as bass
    import concourse.mybir as mybir
    import concourse.tile as tile
    from concourse import bacc
    from contextlib import ExitStack

    F32 = mybir.dt.float32
    F32R = mybir.dt.float32r
    F16 = mybir.dt.float16
    AP = bass.AP
    AF = mybir.ActivationFunctionType
    ALU = mybir.AluOpType

    nc = bacc.Bacc("TRN2", target_bir_lowering=False, debug=False,
                   num_devices=N_CORES)

    # ---------------- DRAM I/O ----------------
    xT = nc.dram_tensor("xT", [D, S], F32, kind="ExternalInput").ap()
    wq = nc.dram_tensor("wq", [D, 256], F32, kind="ExternalInput").ap()
    wk = nc.dram_tensor("wk", [D, 256], F32, kind="ExternalInput").ap()
    wv = nc.dram_tensor("wv", [D, 256], F32, kind="ExternalInput").ap()
    bq = nc.dram_tensor("bq", [128, 2], F32, kind="ExternalInput").ap()
    bk = nc.dram_tensor("bk", [128, 2], F32, kind="ExternalInput").ap()
    bv = nc.dram_tensor("bv", [128, 2], F32, kind="ExternalInput").ap()
    r01 = nc.dram_tensor("r01", [128, 2], F32, kind="ExternalInput").ap()
    relk_pad = nc.dram_tensor("relk_pad", [128, IMW], F16,
                              kind="ExternalInput").ap()
    rvm = nc.dram_tensor("rvm", [128, 64], F16, kind="ExternalInput").ap()
    rvl = nc.dram_tensor("rvl", [1, 64], F16, kind="ExternalInput").ap()
    wout = nc.dram_tensor("wout", [128, 2, 1024], F16, kind="ExternalInput").ap()
    out = nc.dram_tensor("out", [S, D], F32, kind="ExternalOutput").ap()

    # DRAM scratch images (per head, per q-tile blocks)
    imgq_t = nc.dram_tensor("imgq", [NH * NT * 128 * IMW], F16)
    imgw_t = nc.dram_tensor("imgw", [NH * NT * 128 * IMWW], F16)
    imgc_t = nc.dram_tensor("imgc", [NH * NT * 128 * IMWW], F16)
    imgr_t = nc.dram_tensor("imgr", [2 * 4 * 1024], F16)

    QBLK = NT * 128 * IMW        # per-head stride in imgq
    WBLK = NT * 128 * IMWW       # per-head stride in imgw/imgc
    TBLK = 128 * IMWW            # per-tile stride in imgw/imgc

    def qbase(h, t):
        return (h * NT + t) * 128 * IMW

    def wbase(h, t):
        return (h * NT + t) * 128 * IMWW

    with tile.TileContext(nc) as tc, ExitStack() as ctx:
        # ---------------- persistent pool ----------------
        pp = ctx.enter_context(tc.tile_pool(name="persist", bufs=1))
        qkT = []   # per pair: qT16, kW16, kL16, kR16  [128, S] fp16
        for pair in range(2):
            qkT.append({
                "q": pp.tile([128, S], F16, tag=f"qT{pair}", name=f"qT{pair}"),
                "W": pp.tile([128, S], F16, tag=f"kW{pair}", name=f"kW{pair}"),
                "L": pp.tile([128, S], F16, tag=f"kL{pair}", name=f"kL{pair}"),
                "R": pp.tile([128, S], F16, tag=f"kR{pair}", name=f"kR{pair}"),
            })
        v16 = pp.tile([128, NT, 256], F16, tag="v16", name="v16")
        relk_sb = pp.tile([128, IMW], F16, tag="relk", name="relk")
        rvm_sb = pp.tile([128, 64], F16, tag="rvm", name="rvm")
        rvl_sb = pp.tile([1, 64], F16, tag="rvl", name="rvl")
        wout_sb = pp.tile([128, 2, 1024], F16, tag="wout", name="wout")
        bq_sb = pp.tile([128, 2], F32, tag="bq", name="bq")
        bk_sb = pp.tile([128, 2], F32, tag="bk", name="bk")
        bv_sb = pp.tile([128, 2], F32, tag="bv", name="bv")
        r01_sb = pp.tile([128, 2], F32, tag="r01", name="r01")
        ones1 = pp.tile([1, 128], F16, tag="ones1", name="ones1")
        zeros2 = pp.tile([128, 2, 128], F16, tag="zeros2", name="zeros2")
        ctx16all = [pp.tile([128, 4, 512], F16, tag=f"ctxA{p}", name=f"ctxA{p}")
                    for p in range(2)]

        nc.sync.dma_start(relk_sb[:], relk_pad)
        nc.sync.dma_start(rvm_sb[:], rvm)
        nc.sync.dma_start(rvl_sb[:], rvl)
        nc.sync.dma_start(wout_sb[:], wout)
        nc.sync.dma_start(bq_sb[:], bq)
        nc.sync.dma_start(bk_sb[:], bk)
        nc.sync.dma_start(bv_sb[:], bv)
        nc.sync.dma_start(r01_sb[:], r01)
        nc.gpsimd.memset(ones1[:], 1.0)
        nc.gpsimd.memset(zeros2[:], 0.0)

        # pre-phase-1 pool: tiles that let qrel/band overlap phase-1 tail
        pq = ctx.enter_context(tc.tile_pool(name="pq", bufs=1))
        band_sb = pq.tile([128, 2, NT, IMWW], F16, tag="band", name="band",
                          bufs=1)
        qp16_pool = pq

        # single PSUM pool, tags reused across phases:
        #   sc  [128,1024] x2 (4 banks): phase1 q/k, score chunks
        #   ctx [128, 512] x2 (2 banks): phase1 v, AV context
        #   qrp [128, 512] x2 (2 banks): qrel pads, bc broadcast, out-proj
        ps = ctx.enter_context(tc.tile_pool(name="ps", bufs=2, space="PSUM"))

        # ---------------- phase 1: projections ----------------
        with tc.tile_pool(name="p1", bufs=1) as p1:
            xT_sb = p1.tile([128, 8, S], F32R, tag="xT", name="xT")
            wq_sb = p1.tile([128, 8, 256], F32R, tag="wq", name="wq")
            wk_sb = p1.tile([128, 8, 256], F32R, tag="wk", name="wk")
            wv_sb = p1.tile([128, 8, 256], F32R, tag="wv", name="wv")
            nc.sync.dma_start(xT_sb[:], xT.rearrange("(c p) s -> p c s", p=128).bitcast(F32R))
            nc.sync.dma_start(wq_sb[:], wq.rearrange("(c p) n -> p c n", p=128).bitcast(F32R))
            nc.sync.dma_start(wk_sb[:], wk.rearrange("(c p) n -> p c n", p=128).bitcast(F32R))
            nc.sync.dma_start(wv_sb[:], wv.rearrange("(c p) n -> p c n", p=128).bitcast(F32R))

            # q, k (transposed layout [col, s]); dk-outer for weight reuse
            for pair in range(2):
                cols = slice(128 * pair, 128 * pair + 128)
                for proj, wsb, bsb in (("q", wq_sb, bq_sb), ("k", wk_sb, bk_sb)):
                    halves = [ps.tile([128, CW], F32, tag="sc", name=f"p1{proj}{sh}")
                              for sh in range(2)]
                    for dk in range(8):
                        for sh in range(2):
                            for half in range(2):
                                s0 = 1024 * sh + 512 * half
                                nc.tensor.matmul(
                                    halves[sh][:, 512 * half:512 * half + 512],
                                    wsb[:, dk, cols],
                                    xT_sb[:, dk, s0:s0 + 512],
                                    start=(dk == 0), stop=(dk == 7))
                    for sh in range(2):
                        cs = slice(1024 * sh, 1024 * sh + 1024)
                        if proj == "q":
                            nc.vector.tensor_scalar(
                                qkT[pair]["q"][:, cs], halves[sh][:],
                                bq_sb[:, pair:pair + 1], SCALE,
                                op0=ALU.add, op1=ALU.mult)
                        else:
                            nc.vector.tensor_scalar_add(
                                qkT[pair]["W"][:, cs], halves[sh][:],
                                bk_sb[:, pair:pair + 1])
                            nc.vector.tensor_scalar_add(
                                qkT[pair]["L"][:, cs], qkT[pair]["W"][:, cs],
                                r01_sb[:, 0:1])
                            nc.vector.tensor_scalar_add(
                                qkT[pair]["R"][:, cs], qkT[pair]["W"][:, cs],
                                r01_sb[:, 1:2])

            # v (natural layout [s, col])
            for st in range(NT):
                ps_v = ps.tile([128, 512], F32, tag="ctx", name="p1v")
                for dk in range(8):
                    nc.tensor.matmul(
                        ps_v[:, 0:256], xT_sb[:, dk, 128 * st:128 * st + 128],
                        wv_sb[:, dk, :], start=(dk == 0), stop=(dk == 7))
                nc.vector.tensor_copy(v16[:, st, :], ps_v[:, 0:256])

        # ---------------- phase 2: attention ----------------
        p2 = ctx.enter_context(tc.tile_pool(name="p2", bufs=1))

        for pair in range(2):
            # ---- qrel pads: matmul against padded table, write imgq blocks
            for t in range(NT):
                qrps = [ps.tile([128, 512], F32, tag="qrp", name="qrp")
                        for _ in range(2)]
                for h01 in range(2):
                    rs = slice(64 * h01, 64 * h01 + 64)
                    nc.tensor.matmul(
                        qrps[h01][:], qkT[pair]["q"][rs, 128 * t:128 * t + 128],
                        relk_sb[rs, :], start=True, stop=True)
                qp16 = qp16_pool.tile([128, 2, IMW], F16, tag="qp16",
                                      name="qp16", bufs=3)
                for h01 in range(2):
                    nc.vector.tensor_copy(qp16[:, h01, :], qrps[h01][:])
                nc.gpsimd.dma_start(
                    AP(imgq_t, qbase(2 * pair, t),
                       [[IMW, 128], [QBLK, 2], [1, IMW]]),
                    qp16[:])

            # ---- imgw guard zones for t=0 / t=15 (left/right clip cols)
            nc.gpsimd.dma_start(
                AP(imgw_t, wbase(2 * pair, 0),
                   [[IMWW, 128], [WBLK, 2], [1, 128]]),
                zeros2[:])
            nc.gpsimd.dma_start(
                AP(imgw_t, wbase(2 * pair, NT - 1) + 256,
                   [[IMWW, 128], [WBLK, 2], [1, 128]]),
                zeros2[:])

            # ---- batched diagonal band read (per head)
            for h01 in range(2):
                nc.gpsimd.dma_start(
                    band_sb[:, h01, :, :],
                    AP(imgq_t, qbase(2 * pair + h01, 0) + 128,
                       [[IMW - 1, 128], [128 * IMW, NT], [1, IMWW]]))

            # ---- scores / exp / images / AV, go-grouped
            for t in range(NT):
                go, tq = t // 4, t % 4
                i0, wlo, whi = _regions(t)
                woff = i0 - 128            # virtual W start (uniform grid)
                moff = wlo - woff          # 128 for t=0 else 0
                wlen = whi - wlo

                if tq == 0:
                    DS = p2.tile([128, 2, 4, 2], F32, tag="DS", name="DS",
                                 bufs=2)

                # scores: zone pieces, split at 512 (bank) boundaries,
                # then exp (+ per-chunk accum) -> ex
                ex = p2.tile([128, 2, S], F16, tag="ex", name="ex", bufs=3)
                accs = p2.tile([128, 2, 2], F32, tag="accs", name="accs",
                               bufs=4)
                for c in range(2):
                    clo, chi = CW * c, CW * c + CW
                    bounds = sorted({clo, chi, wlo, whi} |
                                    {b for b in range(clo, chi + 1, 512)})
                    bounds = [b for b in bounds if clo <= b <= chi]
                    sc_pair = [ps.tile([128, CW], F32, tag="sc", name="sc")
                               for _ in range(2)]
                    for h01 in range(2):
                        rs = slice(64 * h01, 64 * h01 + 64)
                        for lo, hi in zip(bounds[:-1], bounds[1:]):
                            if lo >= hi:
                                continue
                            mid = (lo + hi) // 2
                            key = "L" if mid < wlo else ("W" if mid < whi else "R")
                            nc.tensor.matmul(
                                sc_pair[h01][:, lo - clo:hi - clo],
                                qkT[pair]["q"][rs, 128 * t:128 * t + 128],
                                qkT[pair][key][rs, lo:hi],
                                start=True, stop=True)
                        # band add on the W-zone overlap of this chunk
                        blo, bhi = max(wlo, clo), min(whi, chi)
                        if blo < bhi:
                            nc.vector.tensor_add(
                                sc_pair[h01][:, blo - clo:bhi - clo],
                                sc_pair[h01][:, blo - clo:bhi - clo],
                                band_sb[:, h01, t, blo - woff:bhi - woff])
                        nc.scalar.activation(
                            ex[:, h01, clo:chi], sc_pair[h01][:],
                            AF.Exp, accum_out=accs[:, h01, c:c + 1])

                # W-zone cumsum (+ guards) -> Cpad; write imgw/imgc blocks
                Cpad = p2.tile([128, 2, IMWW], F16, tag="Cpad", name="Cpad",
                               bufs=3)
                if moff > 0:
                    nc.vector.memset(Cpad[:, :, 0:moff], 0.0)
                for h01 in range(2):
                    nc.vector.tensor_tensor_scan(
                        Cpad[:, h01, moff:moff + wlen],
                        ex[:, h01, wlo:whi], ex[:, h01, wlo:whi], 0.0,
                        op0=ALU.add, op1=ALU.bypass)
                if moff + wlen < IMWW:
                    for h01 in range(2):
                        nc.vector.tensor_copy(
                            Cpad[:, h01, moff + wlen:IMWW],
                            Cpad[:, h01, moff + wlen - 1:moff + wlen]
                            .broadcast_to([128, IMWW - moff - wlen]))
                nc.sync.dma_start(
                    AP(imgw_t, wbase(2 * pair, t) + moff,
                       [[IMWW, 128], [WBLK, 2], [1, wlen]]),
                    ex[:, :, wlo:whi])
                nc.sync.dma_start(
                    AP(imgc_t, wbase(2 * pair, t),
                       [[IMWW, 128], [WBLK, 2], [1, IMWW]]),
                    Cpad[:])

                # D, sL  (sL = D - W_mass - R_mass)
                nc.vector.tensor_tensor(
                    DS[:, :, tq, 0:1], accs[:, :, 0:1], accs[:, :, 1:2],
                    op=ALU.add)
                nc.vector.tensor_tensor(
                    DS[:, :, tq, 1:2], DS[:, :, tq, 0:1],
                    Cpad[:, :, moff + wlen - 1:moff + wlen], op=ALU.subtract)
                if whi < S:
                    massR = p2.tile([128, 2, 1], F32, tag="massR",
                                    name="massR", bufs=4)
                    for h01 in range(2):
                        nc.vector.tensor_reduce(
                            massR[:, h01, :], ex[:, h01, whi:S],
                            axis=mybir.AxisListType.X, op=ALU.add)
                    nc.vector.tensor_tensor(
                        DS[:, :, tq, 1:2], DS[:, :, tq, 1:2], massR[:],
                        op=ALU.subtract)

                # transpose attn rows -> attnT2 [kp, h01, kt, q]
                if tq == 0:
                    attnT2 = p2.tile([128, 2, NT, 512], F16, tag="attnT2",
                                     name="attnT2", bufs=1)
                nc.sync.dma_start_transpose(
                    attnT2[:, :, :, 128 * tq:128 * tq + 128], ex[:])

                # ---------------- go epilogue ----------------
                if tq == 3:
                    # skew reads: arel diagonals + c0/c1 cumsum samples
                    arel = p2.tile([128, 2, 4, 256], F16, tag="arel",
                                   name="arel", bufs=2)
                    cvals = p2.tile([128, 2, 4, 2], F16, tag="cvals",
                                    name="cvals", bufs=2)
                    for h01 in range(2):
                        nc.gpsimd.dma_start(
                            arel[:, h01, :, 1:128],
                            AP(imgw_t, wbase(2 * pair + h01, 4 * go) + 65,
                               [[IMWW + 1, 128], [TBLK, 4], [1, 127]]))
                        for cc in range(2):
                            nc.gpsimd.dma_start(
                                cvals[:, h01, :, cc:cc + 1],
                                AP(imgc_t,
                                   wbase(2 * pair + h01, 4 * go) + 64 + 127 * cc,
                                   [[IMWW + 1, 128], [TBLK, 4]]))
                    # edges: col0 = sL + c0 ; col128 = (D - sL) - c1
                    nc.vector.tensor_tensor(
                        arel[:, :, :, 0:1], DS[:, :, :, 1:2],
                        cvals[:, :, :, 0:1], op=ALU.add)
                    tmp8 = p2.tile([128, 2, 4, 1], F32, tag="tmp8",
                                   name="tmp8", bufs=2)
                    nc.vector.tensor_tensor(
                        tmp8[:], DS[:, :, :, 0:1], DS[:, :, :, 1:2],
                        op=ALU.subtract)
                    nc.vector.tensor_tensor(
                        arel[:, :, :, 128:129], tmp8[:], cvals[:, :, :, 1:2],
                        op=ALU.subtract)

                    # reciprocals -> [1, 2, 512] via DRAM bounce (scatter
                    # then contiguous read; SBUF-side scatter needs >3 dims)
                    recf = p2.tile([128, 2, 4, 1], F32, tag="recf",
                                   name="recf", bufs=2)
                    nc.vector.reciprocal(recf[:], DS[:, :, :, 0:1])
                    rec16 = p2.tile([128, 2, 4, 1], F16, tag="rec16",
                                    name="rec16", bufs=2)
                    nc.vector.tensor_copy(rec16[:], recf[:])
                    rbase = (pair * 4 + go) * 1024
                    nc.gpsimd.dma_start(
                        AP(imgr_t, rbase, [[1, 128], [512, 2], [128, 4]]),
                        rec16[:])
                    recipT = p2.tile([1, 2, 512], F16, tag="recipT",
                                     name="recipT", bufs=2)
                    nc.gpsimd.dma_start(
                        recipT[:], AP(imgr_t, rbase, [[1024, 1], [1, 1024]]))

                    # arel transpose: [q, (h,t,m)] -> [m%128, (h,t,m//128), q]
                    arelT = p2.tile([128, 16, 128], F16, tag="arelT",
                                    name="arelT", bufs=2)
                    nc.sync.dma_start_transpose(arelT[:], arel[:])
                    arelTv = arelT[:].rearrange(
                        "p (h t m) q -> p h t m q", h=2, t=4, m=2)

                    # recip broadcast [1,512] -> [128,512] via K=1 matmul
                    bc_ps = ps.tile([128, 512], F32, tag="qrp", name="bc")
                    for h01 in range(2):
                        nc.tensor.matmul(
                            bc_ps[64 * h01:64 * h01 + 64, :],
                            ones1[0:1, 0:64], recipT[0:1, h01, :],
                            start=True, stop=True,
                            tile_position=(0, 64 * h01))
                    rbc = p2.tile([128, 512], F32, tag="rbc", name="rbc",
                                  bufs=2)
                    nc.vector.tensor_copy(rbc[:], bc_ps[:])

                    # AV + rel-v
                    ctx_ps = ps.tile([128, 512], F32, tag="ctx", name="av")
                    for h01 in range(2):
                        h = 2 * pair + h01
                        cs = slice(64 * h01, 64 * h01 + 64)
                        tp = (0, 64 * h01)
                        for kt in range(NT):
                            nc.tensor.matmul(
                                ctx_ps[cs, :], v16[:, kt, 64 * h:64 * h + 64],
                                attnT2[:, h01, kt, :], start=(kt == 0),
                                stop=False, tile_position=tp)
                        nc.tensor.matmul(
                            ctx_ps[cs, :], rvm_sb[:, :],
                            arelTv[:, h01, :, 0, :], start=False, stop=False,
                            tile_position=tp)
                        nc.tensor.matmul(
                            ctx_ps[cs, :], rvl_sb[0:1, :],
                            arelTv[0:1, h01, :, 1, :], start=False, stop=True,
                            tile_position=tp)

                    ct = ctx16all[pair][:, go, :]
                    nc.vector.tensor_tensor(ct, ctx_ps[:], rbc[:],
                                            op=ALU.mult)
                    nc.vector.tensor_scalar_add(ct, ct,
                                                bv_sb[:, pair:pair + 1])

                    # ---- output projection (after both pairs' ctx for go)
                    if pair == 1:
                        for tq2 in range(4):
                            out_sb = p2.tile([128, 1024], F32, tag="osb",
                                             name="osb", bufs=3)
                            for nch in range(2):
                                op_ps = ps.tile([128, 512], F32, tag="qrp",
                                                name="op")
                                for pr in range(2):
                                    nc.tensor.matmul(
                                        op_ps[:],
                                        ctx16all[pr][:, go,
                                                     128 * tq2:128 * tq2 + 128],
                                        wout_sb[:, pr, 512 * nch:512 * nch + 512],
                                        start=(pr == 0), stop=(pr == 1))
                                nc.vector.tensor_copy(
                                    out_sb[:, 512 * nch:512 * nch + 512],
                                    op_ps[:])
                            r0_ = 512 * go + 128 * tq2
                            nc.gpsimd.dma_start(out[r0_:r0_ + 128, :], out_sb[:])

    nc.compile()
    return nc


def get_nc():
    if "nc" not in _cache:
        _cache["nc"] = _build()
    return _cache["nc"]


def shard_inputs(inputs):
    """Build per-core input maps from full inputs (layout prep only)."""
    x = np.asarray(inputs["x"], np.float32)
    W_qkv = np.asarray(inputs["W_qkv"], np.float32)
    b_qkv = np.asarray(inputs["b_qkv"], np.float32)
    W_out = np.asarray(inputs["W_out"], np.float32)
    rk = np.asarray(inputs["rel_emb_k"], np.float32)
    rv = np.asarray(inputs["rel_emb_v"], np.float32)

    Wq, Wk, Wv = W_qkv[:, 0:D], W_qkv[:, D:2 * D], W_qkv[:, 2 * D:3 * D]
    bqf, bkf, bvf = b_qkv[0:D], b_qkv[D:2 * D], b_qkv[2 * D:3 * D]

    # clip-padded relk table: col c -> rel_emb_k[clip(c - 192, 0, 128)]
    idx = np.clip(np.arange(IMW) - 192, 0, 128)
    pad64 = rk.T[:, idx]                      # [64, IMW]
    relk_pad_host = np.concatenate([pad64, pad64], 0).astype(np.float16)
    rvm_host = rv[0:128].astype(np.float16)
    rvl_host = rv[128:129].astype(np.float16)
    r0 = np.tile(rk[0], 2).reshape(128, 1)
    r1 = np.tile(rk[128], 2).reshape(128, 1)
    r01_host = np.concatenate([r0, r1], 1).astype(np.float32)

    in_maps = []
    for c in range(N_CORES):
        b, g = c // 4, c % 4
        cols = slice(256 * g, 256 * g + 256)
        m = {
            "xT": np.ascontiguousarray(x[b].T),
            "wq": np.ascontiguousarray(Wq[:, cols]),
            "wk": np.ascontiguousarray(Wk[:, cols]),
            "wv": np.ascontiguousarray(Wv[:, cols]),
            "bq": np.ascontiguousarray(bqf[cols].reshape(2, 128).T),
            "bk": np.ascontiguousarray(bkf[cols].reshape(2, 128).T),
            "bv": np.ascontiguousarray(bvf[cols].reshape(2, 128).T),
            "r01": r01_host,
            "relk_pad": relk_pad_host,
            "rvm": rvm_host,
            "rvl": rvl_host,
            "wout": np.ascontiguousarray(
                W_out[cols].reshape(2, 128, 1024).transpose(1, 0, 2)
            ).astype(np.float16),
        }
        in_maps.append(m)
    return in_maps


def unshard_outputs(results, inputs):
    b_out = np.asarray(inputs["b_out"], np.float32)
    out = np.zeros((B, S, D), np.float32)
    for c in range(N_CORES):
        out[c // 4] += results[c]["out"]
    out += b_out[None, None, :]
    return out


def kernel(**inputs):
    from concourse import bass_utils
    nc = get_nc()
    in_maps = shard_inputs(inputs)
    res = bass_utils.run_bass_kernel_spmd(nc, in_maps, list(range(N_CORES)))
    return unshard_outputs(res.results, inputs)


if __name__ == "__main__":
    import json
    rng = np.random.default_rng(0)
    demo = {
        "x": rng.standard_normal((B, S, D)).astype(np.float32),
        "W_qkv": (rng.standard_normal((D, 3 * D)) * 0.02).astype(np.float32),
        "b_qkv": np.zeros(3 * D, np.float32),
        "W_out": (rng.standard_normal((D, D)) * 0.02).astype(np.float32),
        "b_out": np.zeros(D, np.float32),
        "rel_emb_k": (rng.standard_normal((VOC, HD)) * 0.02).astype(np.float32),
        "rel_emb_v": (rng.standard_normal((VOC, HD)) * 0.02).astype(np.float32),
    }
    o = kernel(**demo)
    print(o.shape, float(np.abs(o).max()))


# revision 20
# speedup vs baseline: 1.6095x; 1.1104x over previous
"""Trainium2 Bass kernel for MultiHeadedSelfAttention with Shaw relative
position embeddings (clipped, R=64), sharded over 8 NeuronCores.

Sharding: core c handles batch b = c//4 and head group g = c%4 (4 heads).
Each core computes a partial output  ctx_g @ W_out[256g:256g+256]  for its
batch; the host sums the 4 partials per batch and adds b_out.

v2: phase-2 restructured for fewer DMA instructions (the v1 bottleneck was
the SP sequencer issuing ~550 small DMAs at ~0.7us fixed cost each):
  - qrel pad built by matmul against a host-padded relk_pad table
  - per-(t,pair) image writes / batched per-(pair[,go]) image reads
  - exp in 1024-wide chunks; sL via D - C_W - massR (massR = DVE reduce)
  - fused 2-head transposes; scatter/gather DMAs moved to gpsimd (SWDGE)
"""
import sys

sys.path.insert(0, "/opt/trn_rl_repo")

import numpy as np

B, S, D, H, RR, VOC = 2, 2048, 1024, 16, 64, 129
HD = 64              # head dim
NH = 4               # heads per core
N_CORES = 8
NT = S // 128        # 16 q-tiles of 128
IMW = 512            # qrel image width (clip-padded)
IMWW = 384           # attn/cumsum image width (W-zone grid)
SCALE = 0.125        # 1/sqrt(64)
CW = 1024            # exp chunk width

_cache = {}


def _regions(t):
    """W-zone bounds for q-tile t."""
    i0 = 128 * t
    wlo = max(0, i0 - 128)
    whi = min(S, i0 + 256)
    return i0, wlo, whi


def _build():
    import concourse.bass as bass
    import concourse.mybir as mybir
    import concourse.tile as tile
    from concourse import bacc
    from concourse.masks import make_identity
    from contextlib import ExitStack

    F32 = mybir.dt.float32
    F32R = mybir.dt.float32r
    F16 = mybir.dt.float16
    AP = bass.AP
    AF = mybir.ActivationFunctionType
    ALU = mybir.AluOpType

    nc = bacc.Bacc("TRN2", target_bir_lowering=False, debug=False,
                   num_devices=N_CORES)

    # ---------------- DRAM I/O ----------------
    xT = nc.dram_tensor("xT", [D, S], F32, kind="ExternalInput").ap()
    wq = nc.dram_tensor("wq", [D, 256], F32, kind="ExternalInput").ap()
    wk = nc.dram_tensor("wk", [D, 256], F32, kind="ExternalInput").ap()
    wv = nc.dram_tensor("wv", [D, 256], F32, kind="ExternalInput").ap()
    bq = nc.dram_tensor("bq", [128, 2], F32, kind="ExternalInput").ap()
    bk = nc.dram_tensor("bk", [128, 2], F32, kind="ExternalInput").ap()
    bv = nc.dram_tensor("bv", [128, 2], F32, kind="ExternalInput").ap()
    r01 = nc.dram_tensor("r01", [128, 2], F32, kind="ExternalInput").ap()
    relk_pad = nc.dram_tensor("relk_pad", [128, IMW], F16,
                              kind="ExternalInput").ap()
    rvm = nc.dram_tensor("rvm", [128, 64], F16, kind="ExternalInput").ap()
    rvl = nc.dram_tensor("rvl", [1, 64], F16, kind="ExternalInput").ap()
    wout = nc.dram_tensor("wout", [128, 2, 1024], F16, kind="ExternalInput").ap()
    out = nc.dram_tensor("out", [S, D], F32, kind="ExternalOutput").ap()

    # DRAM scratch images (per head, per q-tile blocks)
    imgq_t = nc.dram_tensor("imgq", [NH * NT * 128 * IMW], F16)
    imgw_t = nc.dram_tensor("imgw", [NH * NT * 128 * IMWW], F16)
    imgc_t = nc.dram_tensor("imgc", [NH * NT * 128 * IMWW], F16)
    imgr_t = nc.dram_tensor("imgr", [2 * 4 * 1024], F16)

    QBLK = NT * 128 * IMW        # per-head stride in imgq
    WBLK = NT * 128 * IMWW       # per-head stride in imgw/imgc
    TBLK = 128 * IMWW            # per-tile stride in imgw/imgc

    def qbase(h, t):
        return (h * NT + t) * 128 * IMW

    def wbase(h, t):
        return (h * NT + t) * 128 * IMWW

    with tile.TileContext(nc) as tc, ExitStack() as ctx:
        # ---------------- persistent pool ----------------
        pp = ctx.enter_context(tc.tile_pool(name="persist", bufs=1))
        qkT = []   # per pair: qT16, kL16, kR16  [128, S] fp16
        for pair in range(2):
            qkT.append({
                "q": pp.tile([128, S], F16, tag=f"qT{pair}", name=f"qT{pair}"),
                "L": pp.tile([128, S], F16, tag=f"kL{pair}", name=f"kL{pair}"),
                "R": pp.tile([128, S], F16, tag=f"kR{pair}", name=f"kR{pair}"),
            })
        v16 = pp.tile([128, NT, 256], F16, tag="v16", name="v16")
        relk_sb = pp.tile([128, IMW], F16, tag="relk", name="relk")
        rvm_sb = pp.tile([128, 64], F16, tag="rvm", name="rvm")
        rvl_sb = pp.tile([1, 64], F16, tag="rvl", name="rvl")
        wout_sb = pp.tile([128, 2, 1024], F16, tag="wout", name="wout")
        bq_sb = pp.tile([128, 2], F32, tag="bq", name="bq")
        bk_sb = pp.tile([128, 2], F32, tag="bk", name="bk")
        bv_sb = pp.tile([128, 2], F32, tag="bv", name="bv")
        r01_sb = pp.tile([128, 2], F32, tag="r01", name="r01")
        ones1 = pp.tile([1, 128], F16, tag="ones1", name="ones1")
        zeros2 = pp.tile([128, 2, 128], F16, tag="zeros2", name="zeros2")
        ident = pp.tile([128, 128], F16, tag="ident", name="ident")
        ctx16all = [pp.tile([128, 4, 512], F16, tag=f"ctxA{p}", name=f"ctxA{p}")
                    for p in range(2)]

        nc.sync.dma_start(relk_sb[:], relk_pad)
        nc.sync.dma_start(rvm_sb[:], rvm)
        nc.sync.dma_start(rvl_sb[:], rvl)
        nc.sync.dma_start(wout_sb[:], wout)
        nc.sync.dma_start(bq_sb[:], bq)
        nc.sync.dma_start(bk_sb[:], bk)
        nc.sync.dma_start(bv_sb[:], bv)
        nc.sync.dma_start(r01_sb[:], r01)
        nc.gpsimd.memset(ones1[:], 1.0)
        nc.gpsimd.memset(zeros2[:], 0.0)
        make_identity(nc, ident[:])

        # pre-phase-1 pool: tiles that let qrel/band overlap phase-1 tail
        pq = ctx.enter_context(tc.tile_pool(name="pq", bufs=1))
        band_sb = pq.tile([128, 2, NT, IMWW], F16, tag="band", name="band",
                          bufs=1)
        qp16_pool = pq

        # single PSUM pool, tags reused across phases:
        #   sc  [128,1024] x2 (4 banks): phase1 q/k, score chunks
        #   ctx [128, 512] x2 (2 banks): phase1 v, AV context
        #   qrp [128, 512] x2 (2 banks): qrel pads, bc broadcast, out-proj
        ps = ctx.enter_context(tc.tile_pool(name="ps", bufs=2, space="PSUM"))

        # ---------------- phase 1: projections ----------------
        with tc.tile_pool(name="p1", bufs=1) as p1:
            xT_sb = p1.tile([128, 8, S], F32R, tag="xT", name="xT")
            wq_sb = p1.tile([128, 8, 256], F32R, tag="wq", name="wq")
            wk_sb = p1.tile([128, 8, 256], F32R, tag="wk", name="wk")
            wv_sb = p1.tile([128, 8, 256], F32R, tag="wv", name="wv")
            nc.sync.dma_start(xT_sb[:], xT.rearrange("(c p) s -> p c s", p=128).bitcast(F32R))
            nc.sync.dma_start(wq_sb[:], wq.rearrange("(c p) n -> p c n", p=128).bitcast(F32R))
            nc.sync.dma_start(wk_sb[:], wk.rearrange("(c p) n -> p c n", p=128).bitcast(F32R))
            nc.sync.dma_start(wv_sb[:], wv.rearrange("(c p) n -> p c n", p=128).bitcast(F32R))

            # q, k (transposed layout [col, s]); dk-outer for weight reuse
            for pair in range(2):
                cols = slice(128 * pair, 128 * pair + 128)
                for proj, wsb, bsb in (("q", wq_sb, bq_sb), ("k", wk_sb, bk_sb)):
                    halves = [ps.tile([128, CW], F32, tag="sc", name=f"p1{proj}{sh}")
                              for sh in range(2)]
                    for dk in range(8):
                        for sh in range(2):
                            for half in range(2):
                                s0 = 1024 * sh + 512 * half
                                nc.tensor.matmul(
                                    halves[sh][:, 512 * half:512 * half + 512],
                                    wsb[:, dk, cols],
                                    xT_sb[:, dk, s0:s0 + 512],
                                    start=(dk == 0), stop=(dk == 7))
                    for sh in range(2):
                        cs = slice(1024 * sh, 1024 * sh + 1024)
                        if proj == "q":
                            nc.vector.tensor_scalar(
                                qkT[pair]["q"][:, cs], halves[sh][:],
                                bq_sb[:, pair:pair + 1], SCALE,
                                op0=ALU.add, op1=ALU.mult)
                        else:
                            nc.vector.tensor_scalar(
                                qkT[pair]["L"][:, cs], halves[sh][:],
                                bk_sb[:, pair:pair + 1], r01_sb[:, 0:1],
                                op0=ALU.add, op1=ALU.add)
                            nc.vector.tensor_scalar(
                                qkT[pair]["R"][:, cs], halves[sh][:],
                                bk_sb[:, pair:pair + 1], r01_sb[:, 1:2],
                                op0=ALU.add, op1=ALU.add)

            # v (natural layout [s, col])
            for st in range(NT):
                ps_v = ps.tile([128, 512], F32, tag="ctx", name="p1v")
                for dk in range(8):
                    nc.tensor.matmul(
                        ps_v[:, 0:256], xT_sb[:, dk, 128 * st:128 * st + 128],
                        wv_sb[:, dk, :], start=(dk == 0), stop=(dk == 7))
                nc.vector.tensor_copy(v16[:, st, :], ps_v[:, 0:256])

        # ---------------- phase 2: attention ----------------
        p2 = ctx.enter_context(tc.tile_pool(name="p2", bufs=1))

        for pair in range(2):
            # ---- qrel pads: matmul against padded table, write imgq blocks
            for t in range(NT):
                qrps = [ps.tile([128, 512], F32, tag="qrp", name="qrp")
                        for _ in range(2)]
                for h01 in range(2):
                    rs = slice(64 * h01, 64 * h01 + 64)
                    nc.tensor.matmul(
                        qrps[h01][:], qkT[pair]["q"][rs, 128 * t:128 * t + 128],
                        relk_sb[rs, :], start=True, stop=True)
                qp16 = qp16_pool.tile([128, 2, IMW], F16, tag="qp16",
                                      name="qp16", bufs=3)
                for h01 in range(2):
                    nc.vector.tensor_copy(qp16[:, h01, :], qrps[h01][:])
                nc.gpsimd.dma_start(
                    AP(imgq_t, qbase(2 * pair, t),
                       [[IMW, 128], [QBLK, 2], [1, IMW]]),
                    qp16[:])

            # ---- imgw guard zones for t=0 / t=15 (left/right clip cols)
            nc.gpsimd.dma_start(
                AP(imgw_t, wbase(2 * pair, 0),
                   [[IMWW, 128], [WBLK, 2], [1, 128]]),
                zeros2[:])
            nc.gpsimd.dma_start(
                AP(imgw_t, wbase(2 * pair, NT - 1) + 256,
                   [[IMWW, 128], [WBLK, 2], [1, 128]]),
                zeros2[:])

            # ---- batched diagonal band read (per head)
            for h01 in range(2):
                nc.gpsimd.dma_start(
                    band_sb[:, h01, :, :],
                    AP(imgq_t, qbase(2 * pair + h01, 0) + 128,
                       [[IMW - 1, 128], [128 * IMW, NT], [1, IMWW]]))

            # ---- scores / exp / images, with AV epilogue delayed 2 t-steps
            pend = {}

            def emit_av(g):
                st = pend.pop(g)
                # AV content first (attnT2 ready ~2 t ago), then rel, bc, ct
                ctx_ps = ps.tile([128, 512], F32, tag="ctx", name="av")
                for h01 in range(2):
                    h = 2 * pair + h01
                    cs = slice(64 * h01, 64 * h01 + 64)
                    tp = (0, 64 * h01)
                    for kt in range(NT):
                        nc.tensor.matmul(
                            ctx_ps[cs, :], v16[:, kt, 64 * h:64 * h + 64],
                            st["attnT2"][:, h01, kt, :], start=(kt == 0),
                            stop=False, tile_position=tp)
                    nc.tensor.matmul(
                        ctx_ps[cs, :], rvm_sb[:, :],
                        st["arelTv"][:, h01, :, 0, :], start=False,
                        stop=False, tile_position=tp)
                    nc.tensor.matmul(
                        ctx_ps[cs, :], rvl_sb[0:1, :],
                        st["arelTv"][0:1, h01, :, 1, :], start=False,
                        stop=True, tile_position=tp)

                # recip broadcast [1,512] -> [128,512] via K=1 matmul
                bc_ps = ps.tile([128, 512], F32, tag="qrp", name="bc")
                for h01 in range(2):
                    nc.tensor.matmul(
                        bc_ps[64 * h01:64 * h01 + 64, :],
                        ones1[0:1, 0:64], st["recipT"][0:1, h01, :],
                        start=True, stop=True, tile_position=(0, 64 * h01))
                rbc = p2.tile([128, 512], F32, tag="rbc", name="rbc",
                              bufs=2)
                nc.vector.tensor_copy(rbc[:], bc_ps[:])

                ct = ctx16all[pair][:, g, :]
                nc.vector.tensor_tensor(ct, ctx_ps[:], rbc[:], op=ALU.mult)
                nc.vector.tensor_scalar_add(ct, ct, bv_sb[:, pair:pair + 1])

                # ---- output projection (after both pairs' ctx for g)
                if pair == 1:
                    for tq2 in range(4):
                        out_sb = p2.tile([128, 1024], F32, tag="osb",
                                         name="osb", bufs=2)
                        for nch in range(2):
                            op_ps = ps.tile([128, 512], F32, tag="qrp",
                                            name="op")
                            for pr in range(2):
                                nc.tensor.matmul(
                                    op_ps[:],
                                    ctx16all[pr][:, g,
                                                 128 * tq2:128 * tq2 + 128],
                                    wout_sb[:, pr, 512 * nch:512 * nch + 512],
                                    start=(pr == 0), stop=(pr == 1))
                            nc.vector.tensor_copy(
                                out_sb[:, 512 * nch:512 * nch + 512],
                                op_ps[:])
                        r0_ = 512 * g + 128 * tq2
                        nc.gpsimd.dma_start(out[r0_:r0_ + 128, :], out_sb[:])

            for t in range(NT):
                go, tq = t // 4, t % 4
                i0, wlo, whi = _regions(t)
                woff = i0 - 128            # virtual W start (uniform grid)
                moff = wlo - woff          # 128 for t=0 else 0
                wlen = whi - wlo

                if tq == 0:
                    DS = p2.tile([128, 2, 4, 2], F32, tag="DS", name="DS",
                                 bufs=2)
                    attnT2 = p2.tile([128, 2, NT, 512], F16, tag="attnT2",
                                     name="attnT2", bufs=2)

                # scores: kL covers L+W (band is re-based), kR to the right;
                # pieces split at 512 (bank) boundaries; band added on PE
                # via identity-matmul accumulate; then exp (+ accum) -> ex
                ex = p2.tile([128, 2, S], F16, tag="ex", name="ex", bufs=3)
                accs = p2.tile([128, 2, 2], F32, tag="accs", name="accs",
                               bufs=4)
                for c in range(2):
                    clo, chi = CW * c, CW * c + CW
                    bounds = sorted({clo, chi, whi} |
                                    {b for b in range(clo, chi + 1, 512)})
                    bounds = [b for b in bounds if clo <= b <= chi]
                    sc_pair = [ps.tile([128, CW], F32, tag="sc", name="sc")
                               for _ in range(2)]
                    for h01 in range(2):
                        rs = slice(64 * h01, 64 * h01 + 64)
                        for lo, hi in zip(bounds[:-1], bounds[1:]):
                            if lo >= hi:
                                continue
                            key = "L" if (lo + hi) // 2 < whi else "R"
                            bl, bh = max(lo, wlo), min(hi, whi)
                            nc.tensor.matmul(
                                sc_pair[h01][:, lo - clo:hi - clo],
                                qkT[pair]["q"][rs, 128 * t:128 * t + 128],
                                qkT[pair][key][rs, lo:hi],
                                start=True, stop=(bl >= bh))
                            if bl < bh:
                                nc.tensor.matmul(
                                    sc_pair[h01][:, bl - clo:bh - clo],
                                    ident[:],
                                    band_sb[:, h01, t, bl - woff:bh - woff],
                                    start=False, stop=True)
                        nc.scalar.activation(
                            ex[:, h01, clo:chi], sc_pair[h01][:],
                            AF.Exp, accum_out=accs[:, h01, c:c + 1])

                # W-zone cumsum (+ guards) -> Cpad; write imgw/imgc blocks
                Cpad = p2.tile([128, 2, IMWW], F16, tag="Cpad", name="Cpad",
                               bufs=2)
                if moff > 0:
                    nc.vector.memset(Cpad[:, :, 0:moff], 0.0)
                for h01 in range(2):
                    nc.vector.tensor_tensor_scan(
                        Cpad[:, h01, moff:moff + wlen],
                        ex[:, h01, wlo:whi], ex[:, h01, wlo:whi], 0.0,
                        op0=ALU.add, op1=ALU.bypass)
                if moff + wlen < IMWW:
                    for h01 in range(2):
                        nc.vector.tensor_copy(
                            Cpad[:, h01, moff + wlen:IMWW],
                            Cpad[:, h01, moff + wlen - 1:moff + wlen]
                            .broadcast_to([128, IMWW - moff - wlen]))
                nc.sync.dma_start(
                    AP(imgw_t, wbase(2 * pair, t) + moff,
                       [[IMWW, 128], [WBLK, 2], [1, wlen]]),
                    ex[:, :, wlo:whi])
                nc.sync.dma_start(
                    AP(imgc_t, wbase(2 * pair, t),
                       [[IMWW, 128], [WBLK, 2], [1, IMWW]]),
                    Cpad[:])

                # D, sL  (sL = D - W_mass - R_mass)
                nc.vector.tensor_tensor(
                    DS[:, :, tq, 0:1], accs[:, :, 0:1], accs[:, :, 1:2],
                    op=ALU.add)
                nc.vector.tensor_tensor(
                    DS[:, :, tq, 1:2], DS[:, :, tq, 0:1],
                    Cpad[:, :, moff + wlen - 1:moff + wlen], op=ALU.subtract)
                if whi < S:
                    massR = p2.tile([128, 2, 1], F32, tag="massR",
                                    name="massR", bufs=4)
                    for h01 in range(2):
                        nc.vector.tensor_reduce(
                            massR[:, h01, :], ex[:, h01, whi:S],
                            axis=mybir.AxisListType.X, op=ALU.add)
                    nc.vector.tensor_tensor(
                        DS[:, :, tq, 1:2], DS[:, :, tq, 1:2], massR[:],
                        op=ALU.subtract)

                # transpose attn rows -> attnT2 [kp, h01, kt, q]
                nc.sync.dma_start_transpose(
                    attnT2[:, :, :, 128 * tq:128 * tq + 128], ex[:])

                # -------- go epilogue part 1 (skew reads, edges, recips)
                if tq == 3:
                    arel = p2.tile([128, 2, 4, 256], F16, tag="arel",
                                   name="arel", bufs=2)
                    cvals = p2.tile([128, 2, 4, 2], F16, tag="cvals",
                                    name="cvals", bufs=2)
                    for h01 in range(2):
                        nc.gpsimd.dma_start(
                            arel[:, h01, :, 1:128],
                            AP(imgw_t, wbase(2 * pair + h01, 4 * go) + 65,
                               [[IMWW + 1, 128], [TBLK, 4], [1, 127]]))
                        for cc in range(2):
                            nc.gpsimd.dma_start(
                                cvals[:, h01, :, cc:cc + 1],
                                AP(imgc_t,
                                   wbase(2 * pair + h01, 4 * go) + 64 + 127 * cc,
                                   [[IMWW + 1, 128], [TBLK, 4]]))
                    # reciprocals -> [1, 2, 512] via DRAM bounce
                    recf = p2.tile([128, 2, 4, 1], F32, tag="recf",
                                   name="recf", bufs=2)
                    nc.vector.reciprocal(recf[:], DS[:, :, :, 0:1])
                    rec16 = p2.tile([128, 2, 4, 1], F16, tag="rec16",
                                    name="rec16", bufs=2)
                    nc.vector.tensor_copy(rec16[:], recf[:])
                    rbase = (pair * 4 + go) * 1024
                    nc.gpsimd.dma_start(
                        AP(imgr_t, rbase, [[1, 128], [512, 2], [128, 4]]),
                        rec16[:])
                    recipT = p2.tile([1, 2, 512], F16, tag="recipT",
                                     name="recipT", bufs=2)
                    nc.gpsimd.dma_start(
                        recipT[:], AP(imgr_t, rbase, [[1024, 1], [1, 1024]]))

                    # edges: col0 = sL + c0 ; col128 = (D - sL) - c1
                    nc.vector.tensor_tensor(
                        arel[:, :, :, 0:1], DS[:, :, :, 1:2],
                        cvals[:, :, :, 0:1], op=ALU.add)
                    tmp8 = p2.tile([128, 2, 4, 1], F32, tag="tmp8",
                                   name="tmp8", bufs=2)
                    nc.vector.tensor_tensor(
                        tmp8[:], DS[:, :, :, 0:1], DS[:, :, :, 1:2],
                        op=ALU.subtract)
                    nc.vector.tensor_tensor(
                        arel[:, :, :, 128:129], tmp8[:], cvals[:, :, :, 1:2],
                        op=ALU.subtract)

                    # arel transpose: [q, (h,t,m)] -> [m%128, (h,t,m//128), q]
                    arelT = p2.tile([128, 16, 128], F16, tag="arelT",
                                    name="arelT", bufs=2)
                    nc.sync.dma_start_transpose(arelT[:], arel[:])
                    pend[go] = {
                        "attnT2": attnT2,
                        "arelTv": arelT[:].rearrange(
                            "p (h t m) q -> p h t m q", h=2, t=4, m=2),
                        "recipT": recipT,
                    }

                # -------- delayed AV epilogue (2 t-steps behind)
                if tq == 1 and t >= 5:
                    emit_av(go - 1)

            emit_av(3)

    nc.compile()
    return nc


def get_nc():
    if "nc" not in _cache:
        _cache["nc"] = _build()
    return _cache["nc"]


def shard_inputs(inputs):
    """Build per-core input maps from full inputs (layout prep only)."""
    x = np.asarray(inputs["x"], np.float32)
    W_qkv = np.asarray(inputs["W_qkv"], np.float32)
    b_qkv = np.asarray(inputs["b_qkv"], np.float32)
    W_out = np.asarray(inputs["W_out"], np.float32)
    rk = np.asarray(inputs["rel_emb_k"], np.float32)
    rv = np.asarray(inputs["rel_emb_v"], np.float32)

    Wq, Wk, Wv = W_qkv[:, 0:D], W_qkv[:, D:2 * D], W_qkv[:, 2 * D:3 * D]
    bqf, bkf, bvf = b_qkv[0:D], b_qkv[D:2 * D], b_qkv[2 * D:3 * D]

    # clip-padded, re-based relk table:
    #   col c -> rel_emb_k[clip(c - 192, 0, 128)] - rel_emb_k[0]
    # (re-based so kL = k + rel_emb_k[0] covers the L and W zones; the
    #  band bias vanishes at the far-left clip)
    idx = np.clip(np.arange(IMW) - 192, 0, 128)
    pad64 = rk.T[:, idx] - rk.T[:, 0:1]       # [64, IMW]
    relk_pad_host = np.concatenate([pad64, pad64], 0).astype(np.float16)
    rvm_host = rv[0:128].astype(np.float16)
    rvl_host = rv[128:129].astype(np.float16)
    r0 = np.tile(rk[0], 2).reshape(128, 1)
    r1 = np.tile(rk[128], 2).reshape(128, 1)
    r01_host = np.concatenate([r0, r1], 1).astype(np.float32)

    in_maps = []
    for c in range(N_CORES):
        b, g = c // 4, c % 4
        cols = slice(256 * g, 256 * g + 256)
        m = {
            "xT": np.ascontiguousarray(x[b].T),
            "wq": np.ascontiguousarray(Wq[:, cols]),
            "wk": np.ascontiguousarray(Wk[:, cols]),
            "wv": np.ascontiguousarray(Wv[:, cols]),
            "bq": np.ascontiguousarray(bqf[cols].reshape(2, 128).T),
            "bk": np.ascontiguousarray(bkf[cols].reshape(2, 128).T),
            "bv": np.ascontiguousarray(bvf[cols].reshape(2, 128).T),
            "r01": r01_host,
            "relk_pad": relk_pad_host,
            "rvm": rvm_host,
            "rvl": rvl_host,
            "wout": np.ascontiguousarray(
                W_out[cols].reshape(2, 128, 1024).transpose(1, 0, 2)
            ).astype(np.float16),
        }
        in_maps.append(m)
    return in_maps


def unshard_outputs(results, inputs):
    b_out = np.asarray(inputs["b_out"], np.float32)
    out = np.zeros((B, S, D), np.float32)
    for c in range(N_CORES):
        out[c // 4] += results[c]["out"]
    out += b_out[None, None, :]
    return out


def kernel(**inputs):
    from concourse import bass_utils
    nc = get_nc()
    in_maps = shard_inputs(inputs)
    res = bass_utils.run_bass_kernel_spmd(nc, in_maps, list(range(N_CORES)))
    return unshard_outputs(res.results, inputs)


if __name__ == "__main__":
    import json
    rng = np.random.default_rng(0)
    demo = {
        "x": rng.standard_normal((B, S, D)).astype(np.float32),
        "W_qkv": (rng.standard_normal((D, 3 * D)) * 0.02).astype(np.float32),
        "b_qkv": np.zeros(3 * D, np.float32),
        "W_out": (rng.standard_normal((D, D)) * 0.02).astype(np.float32),
        "b_out": np.zeros(D, np.float32),
        "rel_emb_k": (rng.standard_normal((VOC, HD)) * 0.02).astype(np.float32),
        "rel_emb_v": (rng.standard_normal((VOC, HD)) * 0.02).astype(np.float32),
    }
    o = kernel(**demo)
    print(o.shape, float(np.abs(o).max()))


# revision 32
# speedup vs baseline: 1.6215x; 1.0075x over previous
"""Trainium2 Bass kernel for MultiHeadedSelfAttention with Shaw relative
position embeddings (clipped, R=64), sharded over 8 NeuronCores.

Sharding: core c handles batch b = c//4 and head group g = c%4 (4 heads).
Each core computes a partial output  ctx_g @ W_out[256g:256g+256]  for its
batch; the host sums the 4 partials per batch and adds b_out.

v2: phase-2 restructured for fewer DMA instructions (the v1 bottleneck was
the SP sequencer issuing ~550 small DMAs at ~0.7us fixed cost each):
  - qrel pad built by matmul against a host-padded relk_pad table
  - per-(t,pair) image writes / batched per-(pair[,go]) image reads
  - exp in 1024-wide chunks; sL via D - C_W - massR (massR = DVE reduce)
  - fused 2-head transposes; scatter/gather DMAs moved to gpsimd (SWDGE)
"""
import sys

sys.path.insert(0, "/opt/trn_rl_repo")

import numpy as np

B, S, D, H, RR, VOC = 2, 2048, 1024, 16, 64, 129
HD = 64              # head dim
NH = 4               # heads per core
N_CORES = 8
NT = S // 128        # 16 q-tiles of 128
IMW = 512            # qrel image width (clip-padded)
IMWW = 384           # attn/cumsum image width (W-zone grid)
SCALE = 0.125        # 1/sqrt(64)
CW = 1024            # exp chunk width

_cache = {}


def _regions(t):
    """W-zone bounds for q-tile t."""
    i0 = 128 * t
    wlo = max(0, i0 - 128)
    whi = min(S, i0 + 256)
    return i0, wlo, whi


def _build():
    import concourse.bass as bass
    import concourse.mybir as mybir
    import concourse.tile as tile
    import bass_rust
    from concourse import bacc
    from concourse.masks import make_identity
    from contextlib import ExitStack

    def dap(base, off, dims):
        """Custom-strided view into a DRAM pool tile (keeps dep tracking)."""
        a = base.copy()
        a.offset = a.offset + off
        a.ap = bass_rust.VecI64Pair([list(d) for d in dims])
        return a

    F32 = mybir.dt.float32
    F32R = mybir.dt.float32r
    F16 = mybir.dt.float16
    AP = bass.AP
    AF = mybir.ActivationFunctionType
    ALU = mybir.AluOpType

    nc = bacc.Bacc("TRN2", target_bir_lowering=False, debug=False,
                   num_devices=N_CORES)

    # ---------------- DRAM I/O ----------------
    xT = nc.dram_tensor("xT", [D, S], F32, kind="ExternalInput").ap()
    wq = nc.dram_tensor("wq", [D, 256], F32, kind="ExternalInput").ap()
    wk = nc.dram_tensor("wk", [D, 256], F32, kind="ExternalInput").ap()
    wv = nc.dram_tensor("wv", [D, 256], F32, kind="ExternalInput").ap()
    bq = nc.dram_tensor("bq", [128, 2], F32, kind="ExternalInput").ap()
    bk = nc.dram_tensor("bk", [128, 2], F32, kind="ExternalInput").ap()
    bv = nc.dram_tensor("bv", [128, 2], F32, kind="ExternalInput").ap()
    r01 = nc.dram_tensor("r01", [128, 2], F32, kind="ExternalInput").ap()
    relk_pad = nc.dram_tensor("relk_pad", [128, IMW], F16,
                              kind="ExternalInput").ap()
    rvm = nc.dram_tensor("rvm", [128, 64], F16, kind="ExternalInput").ap()
    rvl = nc.dram_tensor("rvl", [1, 64], F16, kind="ExternalInput").ap()
    wout = nc.dram_tensor("wout", [128, 2, 1024], F16, kind="ExternalInput").ap()
    out = nc.dram_tensor("out", [S, D], F32, kind="ExternalOutput").ap()

    QBLK = NT * 128 * IMW        # per-head stride in imgq
    WBLK = NT * 128 * IMWW       # per-head stride in imgw/imgc
    TBLK = 128 * IMWW            # per-tile stride in imgw/imgc

    def qbase(h, t):
        return (h * NT + t) * 128 * IMW

    def wbase(h, t):
        return (h * NT + t) * 128 * IMWW

    with tile.TileContext(nc) as tc, ExitStack() as ctx:
        # DRAM scratch images as pool tiles => DMA RAW deps are tracked
        pdram = ctx.enter_context(tc.tile_pool(name="dram", bufs=1,
                                               space="DRAM"))
        imgq_t = pdram.tile([NH * NT * 128 * IMW], F16, tag="imgq",
                            name="imgq")
        imgw_t = pdram.tile([NH * NT * 128 * IMWW], F16, tag="imgw",
                            name="imgw")
        imgc_t = pdram.tile([NH * NT * 128 * IMWW], F16, tag="imgc",
                            name="imgc")
        imgr_t = pdram.tile([2 * 4 * 1024], F16, tag="imgr", name="imgr")

        # ---------------- persistent pool ----------------
        pp = ctx.enter_context(tc.tile_pool(name="persist", bufs=1))
        qkT = []   # per pair: qT16, kL16, kR16  [128, S] fp16
        for pair in range(2):
            qkT.append({
                "q": pp.tile([128, S], F16, tag=f"qT{pair}", name=f"qT{pair}"),
                "L": pp.tile([128, S], F16, tag=f"kL{pair}", name=f"kL{pair}"),
                "R": pp.tile([128, S], F16, tag=f"kR{pair}", name=f"kR{pair}"),
            })
        v16 = pp.tile([128, NT, 256], F16, tag="v16", name="v16")
        relk_sb = pp.tile([128, IMW], F16, tag="relk", name="relk")
        rvm_sb = pp.tile([128, 64], F16, tag="rvm", name="rvm")
        rvl_sb = pp.tile([1, 64], F16, tag="rvl", name="rvl")
        wout_sb = pp.tile([128, 2, 1024], F16, tag="wout", name="wout")
        bq_sb = pp.tile([128, 2], F32, tag="bq", name="bq")
        bk_sb = pp.tile([128, 2], F32, tag="bk", name="bk")
        bv_sb = pp.tile([128, 2], F32, tag="bv", name="bv")
        r01_sb = pp.tile([128, 2], F32, tag="r01", name="r01")
        ones1 = pp.tile([1, 128], F16, tag="ones1", name="ones1")
        zeros2 = pp.tile([128, 2, 128], F16, tag="zeros2", name="zeros2")
        ident = pp.tile([128, 128], F16, tag="ident", name="ident")
        ctx16all = [pp.tile([128, 4, 512], F16, tag=f"ctxA{p}", name=f"ctxA{p}")
                    for p in range(2)]

        nc.sync.dma_start(relk_sb[:], relk_pad)
        nc.sync.dma_start(rvm_sb[:], rvm)
        nc.sync.dma_start(rvl_sb[:], rvl)
        nc.sync.dma_start(wout_sb[:], wout)
        nc.sync.dma_start(bq_sb[:], bq)
        nc.sync.dma_start(bk_sb[:], bk)
        nc.sync.dma_start(bv_sb[:], bv)
        nc.sync.dma_start(r01_sb[:], r01)
        nc.gpsimd.memset(ones1[:], 1.0)
        nc.gpsimd.memset(zeros2[:], 0.0)
        make_identity(nc, ident[:])

        # pre-phase-1 pool: tiles that let qrel/band overlap phase-1 tail
        pq = ctx.enter_context(tc.tile_pool(name="pq", bufs=1))
        band_sb = pq.tile([128, 2, NT, IMWW], F16, tag="band", name="band",
                          bufs=1)
        qp16_pool = pq

        # single PSUM pool, tags reused across phases:
        #   sc  [128,1024] x2 (4 banks): phase1 q/k, score chunks
        #   ctx [128, 512] x2 (2 banks): phase1 v, AV context
        #   qrp [128, 512] x2 (2 banks): qrel pads, bc broadcast, out-proj
        ps = ctx.enter_context(tc.tile_pool(name="ps", bufs=2, space="PSUM"))

        # ---------------- phase 1: projections ----------------
        with tc.tile_pool(name="p1", bufs=1) as p1:
            xT_sb = p1.tile([128, 8, S], F32R, tag="xT", name="xT")
            wq_sb = p1.tile([128, 8, 256], F32R, tag="wq", name="wq")
            wk_sb = p1.tile([128, 8, 256], F32R, tag="wk", name="wk")
            wv_sb = p1.tile([128, 8, 256], F32R, tag="wv", name="wv")
            nc.sync.dma_start(xT_sb[:], xT.rearrange("(c p) s -> p c s", p=128).bitcast(F32R))
            nc.sync.dma_start(wq_sb[:], wq.rearrange("(c p) n -> p c n", p=128).bitcast(F32R))
            nc.sync.dma_start(wk_sb[:], wk.rearrange("(c p) n -> p c n", p=128).bitcast(F32R))
            nc.sync.dma_start(wv_sb[:], wv.rearrange("(c p) n -> p c n", p=128).bitcast(F32R))

            # q, k (transposed layout [col, s]); dk-outer for weight reuse
            for pair in range(2):
                cols = slice(128 * pair, 128 * pair + 128)
                for proj, wsb, bsb in (("q", wq_sb, bq_sb), ("k", wk_sb, bk_sb)):
                    halves = [ps.tile([128, CW], F32, tag="sc", name=f"p1{proj}{sh}")
                              for sh in range(2)]
                    for dk in range(8):
                        for sh in range(2):
                            for half in range(2):
                                s0 = 1024 * sh + 512 * half
                                nc.tensor.matmul(
                                    halves[sh][:, 512 * half:512 * half + 512],
                                    wsb[:, dk, cols],
                                    xT_sb[:, dk, s0:s0 + 512],
                                    start=(dk == 0), stop=(dk == 7))
                    for sh in range(2):
                        cs = slice(1024 * sh, 1024 * sh + 1024)
                        if proj == "q":
                            nc.vector.tensor_scalar(
                                qkT[pair]["q"][:, cs], halves[sh][:],
                                bq_sb[:, pair:pair + 1], SCALE,
                                op0=ALU.add, op1=ALU.mult)
                        else:
                            nc.vector.tensor_scalar(
                                qkT[pair]["L"][:, cs], halves[sh][:],
                                bk_sb[:, pair:pair + 1], r01_sb[:, 0:1],
                                op0=ALU.add, op1=ALU.add)
                            nc.vector.tensor_scalar(
                                qkT[pair]["R"][:, cs], halves[sh][:],
                                bk_sb[:, pair:pair + 1], r01_sb[:, 1:2],
                                op0=ALU.add, op1=ALU.add)

            # qrel pads for both pairs: matmul against padded table,
            # write imgq blocks (overlaps k/v projections above via deps)
            for pair in range(2):
                for t in range(NT):
                    qrps = [ps.tile([128, 512], F32, tag="qrp", name="qrp")
                            for _ in range(2)]
                    for h01 in range(2):
                        rs = slice(64 * h01, 64 * h01 + 64)
                        nc.tensor.matmul(
                            qrps[h01][:],
                            qkT[pair]["q"][rs, 128 * t:128 * t + 128],
                            relk_sb[rs, :], start=True, stop=True)
                    qp16 = qp16_pool.tile([128, 2, IMW], F16, tag="qp16",
                                          name="qp16", bufs=3)
                    for h01 in range(2):
                        nc.vector.tensor_copy(qp16[:, h01, :], qrps[h01][:])
                    nc.gpsimd.dma_start(
                        dap(imgq_t, qbase(2 * pair, t),
                           [[IMW, 128], [QBLK, 2], [1, IMW]]),
                        qp16[:])

            # v (natural layout [s, col])
            for st in range(NT):
                ps_v = ps.tile([128, 512], F32, tag="ctx", name="p1v")
                for dk in range(8):
                    nc.tensor.matmul(
                        ps_v[:, 0:256], xT_sb[:, dk, 128 * st:128 * st + 128],
                        wv_sb[:, dk, :], start=(dk == 0), stop=(dk == 7))
                nc.vector.tensor_copy(v16[:, st, :], ps_v[:, 0:256])

        # ---------------- phase 2: attention ----------------
        p2 = ctx.enter_context(tc.tile_pool(name="p2", bufs=1))

        for pair in range(2):
            # ---- imgw guard zones for t=0 / t=15 (left/right clip cols)
            nc.gpsimd.dma_start(
                dap(imgw_t, wbase(2 * pair, 0),
                   [[IMWW, 128], [WBLK, 2], [1, 128]]),
                zeros2[:])
            nc.gpsimd.dma_start(
                dap(imgw_t, wbase(2 * pair, NT - 1) + 256,
                   [[IMWW, 128], [WBLK, 2], [1, 128]]),
                zeros2[:])

            # ---- batched diagonal band read (per head)
            for h01 in range(2):
                nc.gpsimd.dma_start(
                    band_sb[:, h01, :, :],
                    dap(imgq_t, qbase(2 * pair + h01, 0) + 128,
                       [[IMW - 1, 128], [128 * IMW, NT], [1, IMWW]]))

            # ---- scores / exp / images, with AV epilogue delayed 2 t-steps
            pend = {}

            def emit_av(g):
                st = pend.pop(g)
                # AV content first (attnT2 ready ~2 t ago), then rel, bc, ct
                ctx_ps = ps.tile([128, 512], F32, tag="ctx", name="av")
                for h01 in range(2):
                    h = 2 * pair + h01
                    cs = slice(64 * h01, 64 * h01 + 64)
                    tp = (0, 64 * h01)
                    for kt in range(NT):
                        nc.tensor.matmul(
                            ctx_ps[cs, :], v16[:, kt, 64 * h:64 * h + 64],
                            st["attnT2"][:, h01, kt, :], start=(kt == 0),
                            stop=False, tile_position=tp)
                    nc.tensor.matmul(
                        ctx_ps[cs, :], rvm_sb[:, :],
                        st["arelTv"][:, h01, :, 0, :], start=False,
                        stop=False, tile_position=tp)
                    nc.tensor.matmul(
                        ctx_ps[cs, :], rvl_sb[0:1, :],
                        st["arelTv"][0:1, h01, :, 1, :], start=False,
                        stop=True, tile_position=tp)

                # recip broadcast [1,512] -> [128,512] via K=1 matmul
                bc_ps = ps.tile([128, 512], F32, tag="qrp", name="bc")
                for h01 in range(2):
                    nc.tensor.matmul(
                        bc_ps[64 * h01:64 * h01 + 64, :],
                        ones1[0:1, 0:64], st["recipT"][0:1, h01, :],
                        start=True, stop=True, tile_position=(0, 64 * h01))
                rbc = p2.tile([128, 512], F32, tag="rbc", name="rbc",
                              bufs=2)
                nc.vector.tensor_copy(rbc[:], bc_ps[:])

                ct = ctx16all[pair][:, g, :]
                nc.vector.tensor_tensor(ct, ctx_ps[:], rbc[:], op=ALU.mult)
                nc.vector.tensor_scalar_add(ct, ct, bv_sb[:, pair:pair + 1])

                # ---- output projection (after both pairs' ctx for g)
                if pair == 1:
                    for tq2 in range(4):
                        out_sb = p2.tile([128, 1024], F32, tag="osb",
                                         name="osb", bufs=2)
                        for nch in range(2):
                            op_ps = ps.tile([128, 512], F32, tag="qrp",
                                            name="op")
                            for pr in range(2):
                                nc.tensor.matmul(
                                    op_ps[:],
                                    ctx16all[pr][:, g,
                                                 128 * tq2:128 * tq2 + 128],
                                    wout_sb[:, pr, 512 * nch:512 * nch + 512],
                                    start=(pr == 0), stop=(pr == 1))
                            nc.vector.tensor_copy(
                                out_sb[:, 512 * nch:512 * nch + 512],
                                op_ps[:])
                        r0_ = 512 * g + 128 * tq2
                        nc.gpsimd.dma_start(out[r0_:r0_ + 128, :], out_sb[:])

            for t in range(NT):
                go, tq = t // 4, t % 4
                i0, wlo, whi = _regions(t)
                woff = i0 - 128            # virtual W start (uniform grid)
                moff = wlo - woff          # 128 for t=0 else 0
                wlen = whi - wlo

                if tq == 0:
                    DS = p2.tile([128, 2, 4, 2], F32, tag="DS", name="DS",
                                 bufs=2)
                    attnT2 = p2.tile([128, 2, NT, 512], F16, tag="attnT2",
                                     name="attnT2", bufs=2)

                # scores: kL covers L+W (band is re-based), kR to the right;
                # pieces split at 512 (bank) boundaries; band added on PE
                # via identity-matmul accumulate; then exp (+ accum) -> ex
                ex = p2.tile([128, 2, S], F16, tag="ex", name="ex", bufs=3)
                accs = p2.tile([128, 2, 2], F32, tag="accs", name="accs",
                               bufs=4)
                for c in range(2):
                    clo, chi = CW * c, CW * c + CW
                    bounds = sorted({clo, chi, whi} |
                                    {b for b in range(clo, chi + 1, 512)})
                    bounds = [b for b in bounds if clo <= b <= chi]
                    sc_pair = [ps.tile([128, CW], F32, tag="sc", name="sc")
                               for _ in range(2)]
                    # content pieces interleaved across h01 so the two
                    # row-groups (K rows 0:64 / 64:128) overlap in the array
                    for lo, hi in zip(bounds[:-1], bounds[1:]):
                        if lo >= hi:
                            continue
                        key = "L" if (lo + hi) // 2 < whi else "R"
                        bl, bh = max(lo, wlo), min(hi, whi)
                        for h01 in range(2):
                            rs = slice(64 * h01, 64 * h01 + 64)
                            nc.tensor.matmul(
                                sc_pair[h01][:, lo - clo:hi - clo],
                                qkT[pair]["q"][rs, 128 * t:128 * t + 128],
                                qkT[pair][key][rs, lo:hi],
                                start=True, stop=(bl >= bh))
                        if bl < bh:
                            for h01 in range(2):
                                nc.tensor.matmul(
                                    sc_pair[h01][:, bl - clo:bh - clo],
                                    ident[:],
                                    band_sb[:, h01, t, bl - woff:bh - woff],
                                    start=False, stop=True)
                    for h01 in range(2):
                        nc.scalar.activation(
                            ex[:, h01, clo:chi], sc_pair[h01][:],
                            AF.Exp, accum_out=accs[:, h01, c:c + 1])

                # W-zone cumsum (+ guards) -> Cpad; write imgw/imgc blocks
                Cpad = p2.tile([128, 2, IMWW], F16, tag="Cpad", name="Cpad",
                               bufs=2)
                if moff > 0:
                    nc.vector.memset(Cpad[:, :, 0:moff], 0.0)
                for h01 in range(2):
                    nc.vector.tensor_tensor_scan(
                        Cpad[:, h01, moff:moff + wlen],
                        ex[:, h01, wlo:whi], ex[:, h01, wlo:whi], 0.0,
                        op0=ALU.add, op1=ALU.bypass)
                if moff + wlen < IMWW:
                    for h01 in range(2):
                        nc.vector.tensor_copy(
                            Cpad[:, h01, moff + wlen:IMWW],
                            Cpad[:, h01, moff + wlen - 1:moff + wlen]
                            .broadcast_to([128, IMWW - moff - wlen]))
                nc.sync.dma_start(
                    dap(imgw_t, wbase(2 * pair, t) + moff,
                       [[IMWW, 128], [WBLK, 2], [1, wlen]]),
                    ex[:, :, wlo:whi])
                nc.gpsimd.dma_start(
                    dap(imgc_t, wbase(2 * pair, t),
                       [[IMWW, 128], [WBLK, 2], [1, IMWW]]),
                    Cpad[:])

                # D, sL  (sL = D - W_mass - R_mass)
                nc.vector.tensor_tensor(
                    DS[:, :, tq, 0:1], accs[:, :, 0:1], accs[:, :, 1:2],
                    op=ALU.add)
                nc.vector.tensor_tensor(
                    DS[:, :, tq, 1:2], DS[:, :, tq, 0:1],
                    Cpad[:, :, moff + wlen - 1:moff + wlen], op=ALU.subtract)
                if whi < S:
                    massR = p2.tile([128, 2, 1], F32, tag="massR",
                                    name="massR", bufs=4)
                    for h01 in range(2):
                        nc.vector.tensor_reduce(
                            massR[:, h01, :], ex[:, h01, whi:S],
                            axis=mybir.AxisListType.X, op=ALU.add)
                    nc.vector.tensor_tensor(
                        DS[:, :, tq, 1:2], DS[:, :, tq, 1:2], massR[:],
                        op=ALU.subtract)

                # transpose attn rows -> attnT2 [kp, h01, kt, q]
                nc.sync.dma_start_transpose(
                    attnT2[:, :, :, 128 * tq:128 * tq + 128], ex[:])

                # -------- go epilogue part 1 (skew reads, edges, recips)
                if tq == 3:
                    arel = p2.tile([128, 2, 4, 256], F16, tag="arel",
                                   name="arel", bufs=2)
                    cvals = p2.tile([128, 2, 4, 2], F16, tag="cvals",
                                    name="cvals", bufs=2)
                    for h01 in range(2):
                        nc.gpsimd.dma_start(
                            arel[:, h01, :, 1:128],
                            dap(imgw_t, wbase(2 * pair + h01, 4 * go) + 65,
                               [[IMWW + 1, 128], [TBLK, 4], [1, 127]]))
                        for cc in range(2):
                            nc.gpsimd.dma_start(
                                cvals[:, h01, :, cc:cc + 1],
                                dap(imgc_t,
                                   wbase(2 * pair + h01, 4 * go) + 64 + 127 * cc,
                                   [[IMWW + 1, 128], [TBLK, 4]]))
                    # reciprocals -> [1, 2, 512] via DRAM bounce
                    recf = p2.tile([128, 2, 4, 1], F32, tag="recf",
                                   name="recf", bufs=2)
                    nc.vector.reciprocal(recf[:], DS[:, :, :, 0:1])
                    rec16 = p2.tile([128, 2, 4, 1], F16, tag="rec16",
                                    name="rec16", bufs=2)
                    nc.vector.tensor_copy(rec16[:], recf[:])
                    rbase = (pair * 4 + go) * 1024
                    nc.gpsimd.dma_start(
                        dap(imgr_t, rbase, [[1, 128], [512, 2], [128, 4]]),
                        rec16[:])
                    recipT = p2.tile([1, 2, 512], F16, tag="recipT",
                                     name="recipT", bufs=2)
                    nc.gpsimd.dma_start(
                        recipT[:], dap(imgr_t, rbase, [[1024, 1], [1, 1024]]))

                    # edges: col0 = sL + c0 ; col128 = (D - sL) - c1
                    nc.vector.tensor_tensor(
                        arel[:, :, :, 0:1], DS[:, :, :, 1:2],
                        cvals[:, :, :, 0:1], op=ALU.add)
                    tmp8 = p2.tile([128, 2, 4, 1], F32, tag="tmp8",
                                   name="tmp8", bufs=2)
                    nc.vector.tensor_tensor(
                        tmp8[:], DS[:, :, :, 0:1], DS[:, :, :, 1:2],
                        op=ALU.subtract)
                    nc.vector.tensor_tensor(
                        arel[:, :, :, 128:129], tmp8[:], cvals[:, :, :, 1:2],
                        op=ALU.subtract)

                    # arel transpose: [q, (h,t,m)] -> [m%128, (h,t,m//128), q]
                    arelT = p2.tile([128, 16, 128], F16, tag="arelT",
                                    name="arelT", bufs=2)
                    nc.sync.dma_start_transpose(arelT[:], arel[:])
                    pend[go] = {
                        "attnT2": attnT2,
                        "arelTv": arelT[:].rearrange(
                            "p (h t m) q -> p h t m q", h=2, t=4, m=2),
                        "recipT": recipT,
                    }

                # -------- delayed AV epilogue (2 t-steps behind)
                if tq == 1 and t >= 5:
                    emit_av(go - 1)

            emit_av(3)

    nc.compile()
    return nc


def get_nc():
    if "nc" not in _cache:
        _cache["nc"] = _build()
    return _cache["nc"]


def shard_inputs(inputs):
    """Build per-core input maps from full inputs (layout prep only)."""
    x = np.asarray(inputs["x"], np.float32)
    W_qkv = np.asarray(inputs["W_qkv"], np.float32)
    b_qkv = np.asarray(inputs["b_qkv"], np.float32)
    W_out = np.asarray(inputs["W_out"], np.float32)
    rk = np.asarray(inputs["rel_emb_k"], np.float32)
    rv = np.asarray(inputs["rel_emb_v"], np.float32)

    Wq, Wk, Wv = W_qkv[:, 0:D], W_qkv[:, D:2 * D], W_qkv[:, 2 * D:3 * D]
    bqf, bkf, bvf = b_qkv[0:D], b_qkv[D:2 * D], b_qkv[2 * D:3 * D]

    # clip-padded, re-based relk table:
    #   col c -> rel_emb_k[clip(c - 192, 0, 128)] - rel_emb_k[0]
    # (re-based so kL = k + rel_emb_k[0] covers the L and W zones; the
    #  band bias vanishes at the far-left clip)
    idx = np.clip(np.arange(IMW) - 192, 0, 128)
    pad64 = rk.T[:, idx] - rk.T[:, 0:1]       # [64, IMW]
    relk_pad_host = np.concatenate([pad64, pad64], 0).astype(np.float16)
    rvm_host = rv[0:128].astype(np.float16)
    rvl_host = rv[128:129].astype(np.float16)
    r0 = np.tile(rk[0], 2).reshape(128, 1)
    r1 = np.tile(rk[128], 2).reshape(128, 1)
    r01_host = np.concatenate([r0, r1], 1).astype(np.float32)

    in_maps = []
    for c in range(N_CORES):
        b, g = c // 4, c % 4
        cols = slice(256 * g, 256 * g + 256)
        m = {
            "xT": np.ascontiguousarray(x[b].T),
            "wq": np.ascontiguousarray(Wq[:, cols]),
            "wk": np.ascontiguousarray(Wk[:, cols]),
            "wv": np.ascontiguousarray(Wv[:, cols]),
            "bq": np.ascontiguousarray(bqf[cols].reshape(2, 128).T),
            "bk": np.ascontiguousarray(bkf[cols].reshape(2, 128).T),
            "bv": np.ascontiguousarray(bvf[cols].reshape(2, 128).T),
            "r01": r01_host,
            "relk_pad": relk_pad_host,
            "rvm": rvm_host,
            "rvl": rvl_host,
            "wout": np.ascontiguousarray(
                W_out[cols].reshape(2, 128, 1024).transpose(1, 0, 2)
            ).astype(np.float16),
        }
        in_maps.append(m)
    return in_maps


def unshard_outputs(results, inputs):
    b_out = np.asarray(inputs["b_out"], np.float32)
    out = np.zeros((B, S, D), np.float32)
    for c in range(N_CORES):
        out[c // 4] += results[c]["out"]
    out += b_out[None, None, :]
    return out


def kernel(**inputs):
    from concourse import bass_utils
    nc = get_nc()
    in_maps = shard_inputs(inputs)
    res = bass_utils.run_bass_kernel_spmd(nc, in_maps, list(range(N_CORES)))
    return unshard_outputs(res.results, inputs)


if __name__ == "__main__":
    import json
    rng = np.random.default_rng(0)
    demo = {
        "x": rng.standard_normal((B, S, D)).astype(np.float32),
        "W_qkv": (rng.standard_normal((D, 3 * D)) * 0.02).astype(np.float32),
        "b_qkv": np.zeros(3 * D, np.float32),
        "W_out": (rng.standard_normal((D, D)) * 0.02).astype(np.float32),
        "b_out": np.zeros(D, np.float32),
        "rel_emb_k": (rng.standard_normal((VOC, HD)) * 0.02).astype(np.float32),
        "rel_emb_v": (rng.standard_normal((VOC, HD)) * 0.02).astype(np.float32),
    }
    o = kernel(**demo)
    print(o.shape, float(np.abs(o).max()))


# revision 35
# speedup vs baseline: 1.6607x; 1.0241x over previous
"""Trainium2 Bass kernel for MultiHeadedSelfAttention with Shaw relative
position embeddings (clipped, R=64), sharded over 8 NeuronCores.

Sharding: core c handles batch b = c//4 and head group g = c%4 (4 heads).
Each core computes a partial output  ctx_g @ W_out[256g:256g+256]  for its
batch; the host sums the 4 partials per batch and adds b_out.

v2: phase-2 restructured for fewer DMA instructions (the v1 bottleneck was
the SP sequencer issuing ~550 small DMAs at ~0.7us fixed cost each):
  - qrel pad built by matmul against a host-padded relk_pad table
  - per-(t,pair) image writes / batched per-(pair[,go]) image reads
  - exp in 1024-wide chunks; sL via D - C_W - massR (massR = DVE reduce)
  - fused 2-head transposes; scatter/gather DMAs moved to gpsimd (SWDGE)
"""
import sys

sys.path.insert(0, "/opt/trn_rl_repo")

import numpy as np

B, S, D, H, RR, VOC = 2, 2048, 1024, 16, 64, 129
HD = 64              # head dim
NH = 4               # heads per core
N_CORES = 8
NT = S // 128        # 16 q-tiles of 128
IMW = 512            # qrel image width (clip-padded)
IMWW = 384           # attn/cumsum image width (W-zone grid)
SCALE = 0.125        # 1/sqrt(64)
CW = 1024            # exp chunk width

_cache = {}


def _regions(t):
    """W-zone bounds for q-tile t."""
    i0 = 128 * t
    wlo = max(0, i0 - 128)
    whi = min(S, i0 + 256)
    return i0, wlo, whi


def _build():
    import concourse.bass as bass
    import concourse.mybir as mybir
    import concourse.tile as tile
    import bass_rust
    from concourse import bacc
    from concourse.masks import make_identity
    from contextlib import ExitStack

    def dap(base, off, dims):
        """Custom-strided view into a DRAM pool tile (keeps dep tracking)."""
        a = base.copy()
        a.offset = a.offset + off
        a.ap = bass_rust.VecI64Pair([list(d) for d in dims])
        return a

    F32 = mybir.dt.float32
    F32R = mybir.dt.float32r
    F16 = mybir.dt.float16
    AP = bass.AP
    AF = mybir.ActivationFunctionType
    ALU = mybir.AluOpType

    nc = bacc.Bacc("TRN2", target_bir_lowering=False, debug=False,
                   num_devices=N_CORES)

    # ---------------- DRAM I/O ----------------
    xT = nc.dram_tensor("xT", [D, S], F32, kind="ExternalInput").ap()
    wq = nc.dram_tensor("wq", [D, 256], F32, kind="ExternalInput").ap()
    wk = nc.dram_tensor("wk", [D, 256], F32, kind="ExternalInput").ap()
    wv = nc.dram_tensor("wv", [D, 256], F32, kind="ExternalInput").ap()
    bq = nc.dram_tensor("bq", [128, 2], F32, kind="ExternalInput").ap()
    bk = nc.dram_tensor("bk", [128, 2], F32, kind="ExternalInput").ap()
    bv = nc.dram_tensor("bv", [128, 2], F32, kind="ExternalInput").ap()
    r01 = nc.dram_tensor("r01", [128, 2], F32, kind="ExternalInput").ap()
    relk_pad = nc.dram_tensor("relk_pad", [128, IMW], F16,
                              kind="ExternalInput").ap()
    rvm = nc.dram_tensor("rvm", [128, 64], F16, kind="ExternalInput").ap()
    rvl = nc.dram_tensor("rvl", [1, 64], F16, kind="ExternalInput").ap()
    wout = nc.dram_tensor("wout", [128, 2, 1024], F16, kind="ExternalInput").ap()
    out = nc.dram_tensor("out", [S, D], F32, kind="ExternalOutput").ap()

    QBLK = NT * 128 * IMW        # per-head stride in imgq
    WBLK = NT * 128 * IMWW       # per-head stride in imgw/imgc
    TBLK = 128 * IMWW            # per-tile stride in imgw/imgc

    def qbase(h, t):
        return (h * NT + t) * 128 * IMW

    def wbase(h, t):
        return (h * NT + t) * 128 * IMWW

    with tile.TileContext(nc) as tc, ExitStack() as ctx:
        # DRAM scratch images as pool tiles => DMA RAW deps are tracked
        pdram = ctx.enter_context(tc.tile_pool(name="dram", bufs=1,
                                               space="DRAM"))
        imgq_t = pdram.tile([NH * NT * 128 * IMW], F16, tag="imgq",
                            name="imgq")
        imgw_t = pdram.tile([NH * NT * 128 * IMWW], F16, tag="imgw",
                            name="imgw")
        imgc_t = pdram.tile([NH * NT * 128 * IMWW], F16, tag="imgc",
                            name="imgc")
        imgr_t = pdram.tile([2 * 4 * 1024], F16, tag="imgr", name="imgr")

        # ---------------- persistent pool ----------------
        pp = ctx.enter_context(tc.tile_pool(name="persist", bufs=1))
        qkT = []   # per pair: qT16, kL16, kR16  [128, S] fp16
        for pair in range(2):
            qkT.append({
                "q": pp.tile([128, S], F16, tag=f"qT{pair}", name=f"qT{pair}"),
                "L": pp.tile([128, S], F16, tag=f"kL{pair}", name=f"kL{pair}"),
                "R": pp.tile([128, S], F16, tag=f"kR{pair}", name=f"kR{pair}"),
            })
        v16 = pp.tile([128, NT, 256], F16, tag="v16", name="v16")
        relk_sb = pp.tile([128, IMW], F16, tag="relk", name="relk")
        rvm_sb = pp.tile([128, 64], F16, tag="rvm", name="rvm")
        rvl_sb = pp.tile([1, 64], F16, tag="rvl", name="rvl")
        wout_sb = pp.tile([128, 2, 1024], F16, tag="wout", name="wout")
        bq_sb = pp.tile([128, 2], F32, tag="bq", name="bq")
        bk_sb = pp.tile([128, 2], F32, tag="bk", name="bk")
        bv_sb = pp.tile([128, 2], F32, tag="bv", name="bv")
        r01_sb = pp.tile([128, 2], F32, tag="r01", name="r01")
        ones1 = pp.tile([1, 128], F16, tag="ones1", name="ones1")
        zeros2 = pp.tile([128, 2, 128], F16, tag="zeros2", name="zeros2")
        ident = pp.tile([128, 128], F16, tag="ident", name="ident")
        ctx16all = [pp.tile([128, 4, 512], F16, tag=f"ctxA{p}", name=f"ctxA{p}")
                    for p in range(2)]

        nc.sync.dma_start(relk_sb[:], relk_pad)
        nc.sync.dma_start(rvm_sb[:], rvm)
        nc.sync.dma_start(rvl_sb[:], rvl)
        nc.sync.dma_start(wout_sb[:], wout)
        nc.sync.dma_start(bq_sb[:], bq)
        nc.sync.dma_start(bk_sb[:], bk)
        nc.sync.dma_start(bv_sb[:], bv)
        nc.sync.dma_start(r01_sb[:], r01)
        nc.gpsimd.memset(ones1[:], 1.0)
        nc.gpsimd.memset(zeros2[:], 0.0)
        make_identity(nc, ident[:])

        # pre-phase-1 pool: tiles that let qrel/band overlap phase-1 tail
        pq = ctx.enter_context(tc.tile_pool(name="pq", bufs=1))
        band_sb = pq.tile([128, 2, NT, IMWW], F16, tag="band", name="band",
                          bufs=1)
        qp16_pool = pq

        # single PSUM pool, tags reused across phases:
        #   sc  [128,1024] x2 (4 banks): phase1 q/k, score chunks
        #   ctx [128, 512] x2 (2 banks): phase1 v, AV context
        #   qrp [128, 512] x2 (2 banks): qrel pads, bc broadcast, out-proj
        ps = ctx.enter_context(tc.tile_pool(name="ps", bufs=2, space="PSUM"))

        # ---------------- phase 1: projections ----------------
        with tc.tile_pool(name="p1", bufs=1) as p1:
            xT_sb = p1.tile([128, 8, S], F32R, tag="xT", name="xT")
            wq_sb = p1.tile([128, 8, 256], F32R, tag="wq", name="wq")
            wk_sb = p1.tile([128, 8, 256], F32R, tag="wk", name="wk")
            wv_sb = p1.tile([128, 8, 256], F32R, tag="wv", name="wv")
            nc.sync.dma_start(xT_sb[:], xT.rearrange("(c p) s -> p c s", p=128).bitcast(F32R))
            nc.sync.dma_start(wq_sb[:], wq.rearrange("(c p) n -> p c n", p=128).bitcast(F32R))
            nc.sync.dma_start(wk_sb[:], wk.rearrange("(c p) n -> p c n", p=128).bitcast(F32R))
            nc.sync.dma_start(wv_sb[:], wv.rearrange("(c p) n -> p c n", p=128).bitcast(F32R))

            # q, k (transposed layout [col, s]); dk-outer for weight reuse
            for pair in range(2):
                cols = slice(128 * pair, 128 * pair + 128)
                for proj, wsb, bsb in (("q", wq_sb, bq_sb), ("k", wk_sb, bk_sb)):
                    halves = [ps.tile([128, CW], F32, tag="sc", name=f"p1{proj}{sh}")
                              for sh in range(2)]
                    for dk in range(8):
                        for sh in range(2):
                            for half in range(2):
                                s0 = 1024 * sh + 512 * half
                                nc.tensor.matmul(
                                    halves[sh][:, 512 * half:512 * half + 512],
                                    wsb[:, dk, cols],
                                    xT_sb[:, dk, s0:s0 + 512],
                                    start=(dk == 0), stop=(dk == 7))
                    for sh in range(2):
                        cs = slice(1024 * sh, 1024 * sh + 1024)
                        if proj == "q":
                            nc.vector.tensor_scalar(
                                qkT[pair]["q"][:, cs], halves[sh][:],
                                bq_sb[:, pair:pair + 1], SCALE,
                                op0=ALU.add, op1=ALU.mult)
                        else:
                            nc.vector.tensor_scalar(
                                qkT[pair]["L"][:, cs], halves[sh][:],
                                bk_sb[:, pair:pair + 1], r01_sb[:, 0:1],
                                op0=ALU.add, op1=ALU.add)
                            nc.vector.tensor_scalar(
                                qkT[pair]["R"][:, cs], halves[sh][:],
                                bk_sb[:, pair:pair + 1], r01_sb[:, 1:2],
                                op0=ALU.add, op1=ALU.add)

            # qrel pads for both pairs: matmul against padded table,
            # write imgq blocks (overlaps k/v projections above via deps)
            for pair in range(2):
                for t in range(NT):
                    qrps = [ps.tile([128, 512], F32, tag="qrp", name="qrp")
                            for _ in range(2)]
                    for h01 in range(2):
                        rs = slice(64 * h01, 64 * h01 + 64)
                        nc.tensor.matmul(
                            qrps[h01][:],
                            qkT[pair]["q"][rs, 128 * t:128 * t + 128],
                            relk_sb[rs, :], start=True, stop=True)
                    qp16 = qp16_pool.tile([128, 2, IMW], F16, tag="qp16",
                                          name="qp16", bufs=3)
                    for h01 in range(2):
                        nc.vector.tensor_copy(qp16[:, h01, :], qrps[h01][:])
                    nc.gpsimd.dma_start(
                        dap(imgq_t, qbase(2 * pair, t),
                           [[IMW, 128], [QBLK, 2], [1, IMW]]),
                        qp16[:])

            # v (natural layout [s, col])
            for st in range(NT):
                ps_v = ps.tile([128, 512], F32, tag="ctx", name="p1v")
                for dk in range(8):
                    nc.tensor.matmul(
                        ps_v[:, 0:256], xT_sb[:, dk, 128 * st:128 * st + 128],
                        wv_sb[:, dk, :], start=(dk == 0), stop=(dk == 7))
                nc.vector.tensor_copy(v16[:, st, :], ps_v[:, 0:256])

        # ---------------- phase 2: attention ----------------
        p2 = ctx.enter_context(tc.tile_pool(name="p2", bufs=1))

        for pair in range(2):
            # ---- imgw guard zones for t=0 / t=15 (left/right clip cols)
            nc.gpsimd.dma_start(
                dap(imgw_t, wbase(2 * pair, 0),
                   [[IMWW, 128], [WBLK, 2], [1, 128]]),
                zeros2[:])
            nc.gpsimd.dma_start(
                dap(imgw_t, wbase(2 * pair, NT - 1) + 256,
                   [[IMWW, 128], [WBLK, 2], [1, 128]]),
                zeros2[:])

            # ---- batched diagonal band read (per head)
            for h01 in range(2):
                nc.gpsimd.dma_start(
                    band_sb[:, h01, :, :],
                    dap(imgq_t, qbase(2 * pair + h01, 0) + 128,
                       [[IMW - 1, 128], [128 * IMW, NT], [1, IMWW]]))

            # ---- scores / exp / images, with AV epilogue delayed 2 t-steps
            pend = {}

            def emit_av(g):
                st = pend.pop(g)
                # AV content first (attnT2 ready ~2 t ago), then rel, bc, ct
                ctx_ps = ps.tile([128, 512], F32, tag="ctx", name="av")
                for h01 in range(2):
                    h = 2 * pair + h01
                    cs = slice(64 * h01, 64 * h01 + 64)
                    tp = (0, 64 * h01)
                    for kt in range(NT):
                        nc.tensor.matmul(
                            ctx_ps[cs, :], v16[:, kt, 64 * h:64 * h + 64],
                            st["attnT2"][:, h01, kt, :], start=(kt == 0),
                            stop=False, tile_position=tp)
                    nc.tensor.matmul(
                        ctx_ps[cs, :], rvm_sb[:, :],
                        st["arelTv"][:, h01, :, 0, :], start=False,
                        stop=False, tile_position=tp)
                    nc.tensor.matmul(
                        ctx_ps[cs, :], rvl_sb[0:1, :],
                        st["arelTv"][0:1, h01, :, 1, :], start=False,
                        stop=True, tile_position=tp)

                # recip broadcast [1,512] -> [128,512] via K=1 matmul
                bc_ps = ps.tile([128, 512], F32, tag="qrp", name="bc")
                for h01 in range(2):
                    nc.tensor.matmul(
                        bc_ps[64 * h01:64 * h01 + 64, :],
                        ones1[0:1, 0:64], st["recipT"][0:1, h01, :],
                        start=True, stop=True, tile_position=(0, 64 * h01))
                rbc = p2.tile([128, 512], F32, tag="rbc", name="rbc",
                              bufs=2)
                nc.vector.tensor_copy(rbc[:], bc_ps[:])

                ct = ctx16all[pair][:, g, :]
                nc.vector.tensor_tensor(ct, ctx_ps[:], rbc[:], op=ALU.mult)
                nc.vector.tensor_scalar_add(ct, ct, bv_sb[:, pair:pair + 1])

                # ---- output projection (after both pairs' ctx for g)
                if pair == 1:
                    for tq2 in range(4):
                        out_sb = p2.tile([128, 1024], F32, tag="osb",
                                         name="osb", bufs=2)
                        for nch in range(2):
                            op_ps = ps.tile([128, 512], F32, tag="qrp",
                                            name="op")
                            for pr in range(2):
                                nc.tensor.matmul(
                                    op_ps[:],
                                    ctx16all[pr][:, g,
                                                 128 * tq2:128 * tq2 + 128],
                                    wout_sb[:, pr, 512 * nch:512 * nch + 512],
                                    start=(pr == 0), stop=(pr == 1))
                            nc.vector.tensor_copy(
                                out_sb[:, 512 * nch:512 * nch + 512],
                                op_ps[:])
                        r0_ = 512 * g + 128 * tq2
                        nc.gpsimd.dma_start(out[r0_:r0_ + 128, :], out_sb[:])

            for t in range(NT):
                go, tq = t // 4, t % 4
                i0, wlo, whi = _regions(t)
                woff = i0 - 128            # virtual W start (uniform grid)
                moff = wlo - woff          # 128 for t=0 else 0
                wlen = whi - wlo

                if tq == 0:
                    DS = p2.tile([128, 2, 4, 2], F32, tag="DS", name="DS",
                                 bufs=2)
                    attnT2 = p2.tile([128, 2, NT, 512], F16, tag="attnT2",
                                     name="attnT2", bufs=2)

                # scores: kL covers L+W (band is re-based), kR to the right;
                # pieces split at 512 (bank) boundaries; band added on PE
                # via identity-matmul accumulate; then exp (+ accum) -> ex
                ex = p2.tile([128, 2, S], F16, tag="ex", name="ex", bufs=3)
                accs = p2.tile([128, 2, 2], F32, tag="accs", name="accs",
                               bufs=4)
                for c in range(2):
                    clo, chi = CW * c, CW * c + CW
                    bounds = sorted({clo, chi, whi} |
                                    {b for b in range(clo, chi + 1, 512)})
                    bounds = [b for b in bounds if clo <= b <= chi]
                    sc_pair = [ps.tile([128, CW], F32, tag="sc", name="sc")
                               for _ in range(2)]
                    # content pieces interleaved across h01 so the two
                    # row-groups (K rows 0:64 / 64:128) overlap in the array
                    for lo, hi in zip(bounds[:-1], bounds[1:]):
                        if lo >= hi:
                            continue
                        key = "L" if (lo + hi) // 2 < whi else "R"
                        bl, bh = max(lo, wlo), min(hi, whi)
                        for h01 in range(2):
                            rs = slice(64 * h01, 64 * h01 + 64)
                            nc.tensor.matmul(
                                sc_pair[h01][:, lo - clo:hi - clo],
                                qkT[pair]["q"][rs, 128 * t:128 * t + 128],
                                qkT[pair][key][rs, lo:hi],
                                start=True, stop=(bl >= bh))
                        if bl < bh:
                            for h01 in range(2):
                                nc.tensor.matmul(
                                    sc_pair[h01][:, bl - clo:bh - clo],
                                    ident[:],
                                    band_sb[:, h01, t, bl - woff:bh - woff],
                                    start=False, stop=True)
                    for h01 in range(2):
                        nc.scalar.activation(
                            ex[:, h01, clo:chi], sc_pair[h01][:],
                            AF.Exp, accum_out=accs[:, h01, c:c + 1])

                # transpose attn rows -> attnT2 [kp, h01, kt, q]
                nc.sync.dma_start_transpose(
                    attnT2[:, :, :, 128 * tq:128 * tq + 128], ex[:])

                # W-zone cumsum (+ guards) -> Cpad; write imgw/imgc blocks
                Cpad = p2.tile([128, 2, IMWW], F16, tag="Cpad", name="Cpad",
                               bufs=3)
                if moff > 0:
                    nc.vector.memset(Cpad[:, :, 0:moff], 0.0)
                for h01 in range(2):
                    nc.vector.tensor_tensor_scan(
                        Cpad[:, h01, moff:moff + wlen],
                        ex[:, h01, wlo:whi], ex[:, h01, wlo:whi], 0.0,
                        op0=ALU.add, op1=ALU.bypass)
                if moff + wlen < IMWW:
                    for h01 in range(2):
                        nc.vector.tensor_copy(
                            Cpad[:, h01, moff + wlen:IMWW],
                            Cpad[:, h01, moff + wlen - 1:moff + wlen]
                            .broadcast_to([128, IMWW - moff - wlen]))
                nc.sync.dma_start(
                    dap(imgw_t, wbase(2 * pair, t) + moff,
                       [[IMWW, 128], [WBLK, 2], [1, wlen]]),
                    ex[:, :, wlo:whi])
                nc.gpsimd.dma_start(
                    dap(imgc_t, wbase(2 * pair, t),
                       [[IMWW, 128], [WBLK, 2], [1, IMWW]]),
                    Cpad[:])

                # D, sL  (sL = D - W_mass - R_mass)
                nc.vector.tensor_tensor(
                    DS[:, :, tq, 0:1], accs[:, :, 0:1], accs[:, :, 1:2],
                    op=ALU.add)
                nc.vector.tensor_tensor(
                    DS[:, :, tq, 1:2], DS[:, :, tq, 0:1],
                    Cpad[:, :, moff + wlen - 1:moff + wlen], op=ALU.subtract)
                if whi < S:
                    massR = p2.tile([128, 2, 1], F32, tag="massR",
                                    name="massR", bufs=4)
                    for h01 in range(2):
                        nc.vector.tensor_reduce(
                            massR[:, h01, :], ex[:, h01, whi:S],
                            axis=mybir.AxisListType.X, op=ALU.add)
                    nc.vector.tensor_tensor(
                        DS[:, :, tq, 1:2], DS[:, :, tq, 1:2], massR[:],
                        op=ALU.subtract)

                # -------- go epilogue part 1 (skew reads, edges, recips)
                if tq == 3:
                    arel = p2.tile([128, 2, 4, 256], F16, tag="arel",
                                   name="arel", bufs=2)
                    cvals = p2.tile([128, 2, 4, 2], F16, tag="cvals",
                                    name="cvals", bufs=2)
                    for h01 in range(2):
                        nc.gpsimd.dma_start(
                            arel[:, h01, :, 1:128],
                            dap(imgw_t, wbase(2 * pair + h01, 4 * go) + 65,
                               [[IMWW + 1, 128], [TBLK, 4], [1, 127]]))
                        for cc in range(2):
                            nc.gpsimd.dma_start(
                                cvals[:, h01, :, cc:cc + 1],
                                dap(imgc_t,
                                   wbase(2 * pair + h01, 4 * go) + 64 + 127 * cc,
                                   [[IMWW + 1, 128], [TBLK, 4]]))
                    # reciprocals -> [1, 2, 512] via DRAM bounce
                    recf = p2.tile([128, 2, 4, 1], F32, tag="recf",
                                   name="recf", bufs=2)
                    nc.vector.reciprocal(recf[:], DS[:, :, :, 0:1])
                    rec16 = p2.tile([128, 2, 4, 1], F16, tag="rec16",
                                    name="rec16", bufs=2)
                    nc.vector.tensor_copy(rec16[:], recf[:])
                    rbase = (pair * 4 + go) * 1024
                    nc.gpsimd.dma_start(
                        dap(imgr_t, rbase, [[1, 128], [512, 2], [128, 4]]),
                        rec16[:])
                    recipT = p2.tile([1, 2, 512], F16, tag="recipT",
                                     name="recipT", bufs=2)
                    nc.gpsimd.dma_start(
                        recipT[:], dap(imgr_t, rbase, [[1024, 1], [1, 1024]]))

                    # edges: col0 = sL + c0 ; col128 = (D - sL) - c1
                    nc.vector.tensor_tensor(
                        arel[:, :, :, 0:1], DS[:, :, :, 1:2],
                        cvals[:, :, :, 0:1], op=ALU.add)
                    tmp8 = p2.tile([128, 2, 4, 1], F32, tag="tmp8",
                                   name="tmp8", bufs=2)
                    nc.vector.tensor_tensor(
                        tmp8[:], DS[:, :, :, 0:1], DS[:, :, :, 1:2],
                        op=ALU.subtract)
                    nc.vector.tensor_tensor(
                        arel[:, :, :, 128:129], tmp8[:], cvals[:, :, :, 1:2],
                        op=ALU.subtract)

                    # arel transpose: [q, (h,t,m)] -> [m%128, (h,t,m//128), q]
                    arelT = p2.tile([128, 16, 128], F16, tag="arelT",
                                    name="arelT", bufs=2)
                    nc.sync.dma_start_transpose(arelT[:], arel[:])
                    pend[go] = {
                        "attnT2": attnT2,
                        "arelTv": arelT[:].rearrange(
                            "p (h t m) q -> p h t m q", h=2, t=4, m=2),
                        "recipT": recipT,
                    }

                # -------- delayed AV epilogue (2 t-steps behind)
                if tq == 1 and t >= 5:
                    emit_av(go - 1)

            emit_av(3)

    nc.compile()
    return nc


def get_nc():
    if "nc" not in _cache:
        _cache["nc"] = _build()
    return _cache["nc"]


def shard_inputs(inputs):
    """Build per-core input maps from full inputs (layout prep only)."""
    x = np.asarray(inputs["x"], np.float32)
    W_qkv = np.asarray(inputs["W_qkv"], np.float32)
    b_qkv = np.asarray(inputs["b_qkv"], np.float32)
    W_out = np.asarray(inputs["W_out"], np.float32)
    rk = np.asarray(inputs["rel_emb_k"], np.float32)
    rv = np.asarray(inputs["rel_emb_v"], np.float32)

    Wq, Wk, Wv = W_qkv[:, 0:D], W_qkv[:, D:2 * D], W_qkv[:, 2 * D:3 * D]
    bqf, bkf, bvf = b_qkv[0:D], b_qkv[D:2 * D], b_qkv[2 * D:3 * D]

    # clip-padded, re-based relk table:
    #   col c -> rel_emb_k[clip(c - 192, 0, 128)] - rel_emb_k[0]
    # (re-based so kL = k + rel_emb_k[0] covers the L and W zones; the
    #  band bias vanishes at the far-left clip)
    idx = np.clip(np.arange(IMW) - 192, 0, 128)
    pad64 = rk.T[:, idx] - rk.T[:, 0:1]       # [64, IMW]
    relk_pad_host = np.concatenate([pad64, pad64], 0).astype(np.float16)
    rvm_host = rv[0:128].astype(np.float16)
    rvl_host = rv[128:129].astype(np.float16)
    r0 = np.tile(rk[0], 2).reshape(128, 1)
    r1 = np.tile(rk[128], 2).reshape(128, 1)
    r01_host = np.concatenate([r0, r1], 1).astype(np.float32)

    in_maps = []
    for c in range(N_CORES):
        b, g = c // 4, c % 4
        cols = slice(256 * g, 256 * g + 256)
        m = {
            "xT": np.ascontiguousarray(x[b].T),
            "wq": np.ascontiguousarray(Wq[:, cols]),
            "wk": np.ascontiguousarray(Wk[:, cols]),
            "wv": np.ascontiguousarray(Wv[:, cols]),
            "bq": np.ascontiguousarray(bqf[cols].reshape(2, 128).T),
            "bk": np.ascontiguousarray(bkf[cols].reshape(2, 128).T),
            "bv": np.ascontiguousarray(bvf[cols].reshape(2, 128).T),
            "r01": r01_host,
            "relk_pad": relk_pad_host,
            "rvm": rvm_host,
            "rvl": rvl_host,
            "wout": np.ascontiguousarray(
                W_out[cols].reshape(2, 128, 1024).transpose(1, 0, 2)
            ).astype(np.float16),
        }
        in_maps.append(m)
    return in_maps


def unshard_outputs(results, inputs):
    b_out = np.asarray(inputs["b_out"], np.float32)
    out = np.zeros((B, S, D), np.float32)
    for c in range(N_CORES):
        out[c // 4] += results[c]["out"]
    out += b_out[None, None, :]
    return out


def kernel(**inputs):
    from concourse import bass_utils
    nc = get_nc()
    in_maps = shard_inputs(inputs)
    res = bass_utils.run_bass_kernel_spmd(nc, in_maps, list(range(N_CORES)))
    return unshard_outputs(res.results, inputs)


if __name__ == "__main__":
    import json
    rng = np.random.default_rng(0)
    demo = {
        "x": rng.standard_normal((B, S, D)).astype(np.float32),
        "W_qkv": (rng.standard_normal((D, 3 * D)) * 0.02).astype(np.float32),
        "b_qkv": np.zeros(3 * D, np.float32),
        "W_out": (rng.standard_normal((D, D)) * 0.02).astype(np.float32),
        "b_out": np.zeros(D, np.float32),
        "rel_emb_k": (rng.standard_normal((VOC, HD)) * 0.02).astype(np.float32),
        "rel_emb_v": (rng.standard_normal((VOC, HD)) * 0.02).astype(np.float32),
    }
    o = kernel(**demo)
    print(o.shape, float(np.abs(o).max()))
